# revision 15
# baseline (speedup 1.0000x reference)
"""GAT (2-layer, 8-head) Trainium2 kernel over 8 NeuronCores.

Strategy (edge-cut node sharding), v2:
- Pad N 50000->50176 = 8 shards * 6272. Core c owns nodes [6272c, 6272(c+1)).
- Host: sort edges by dest, bucket into 128-node blocks. Within a block, edges
  are split lo (shard-row < HSPL) / hi, because the batched DMA_GATHER takes
  int16 indices (<=32767) and the full table has 50176 rows. HSPL=2688 is
  128-aligned so the A/B table sections align to 128-row tiles.
- Table-1 row: [Wh in (f,h) head-MINOR order 512 | f_src f16 8 | f_src
  residual 8 | pad -> 640] (1280B, elem %256B). Head-minor order makes the
  per-edge attention scale R = p (*) Wh a DVE TENSOR_TENSOR with an OUTER-dim
  broadcast and innermost stride 1 -> 2x_1P perf mode (the head-major layout
  forces a stride-0 innermost broadcast = 1x).
- One dma_gather per (block, section) (fixed SWDGE overhead ~1us dominates,
  so fewer+bigger gathers), round-robin over 4 queues.
- One-hots (OH: [edge,dst], OHT: [dst,edge]) shipped as fp8e4 (0/1 exact):
  halves their HBM traffic and speeds LDWEIGHTS via FWL.
- leaky-relu on the Scalar engine via Prelu(alpha) (same act table set as Exp
  and Copy -> no table reloads); ELU via hm=Exp(ha) then min/add tensor_scalar
  + max (exp is monotone: e^min(x,0) == min(e^x, 1)).
- AllGathers chunked into 3 pieces per table section and interleaved into the
  producing phase so the collective overlaps compute.
- Softmax needs no segment-max: logits are O(6) so exp never overflows, and
  normalization commutes with the scatter-sum (divide once per node).
"""
import os
import sys
sys.path.insert(0, "/opt/trn_rl_repo")
import numpy as np

import concourse.tile as tile
from concourse import bass, bacc, mybir
from concourse.bass_utils import run_bass_kernel_spmd
from concourse.masks import make_identity

N, E = 50000, 800000
NFEAT, NHID, NHEADS, NCLASS = 512, 64, 8, 64
ALPHA = 0.2
NC = 8
NPAD = 50176
SHARD = NPAD // NC        # 6272
BLK = 128
NBPC = SHARD // BLK       # 49 blocks per core
KT = NFEAT // 128         # 4 k-tiles
DW1 = NFEAT + 16          # 528: Wh | src | src_residual
TW1 = 640                 # table-1 row (1280B, %256B for dma_gather)
DW2 = NCLASS + 2          # 66:  Wh2 | src | src_residual
TW2 = 128                 # table-2 row (256B)
HSPL = 2688               # 21*128; section A rows per shard (8*2688<=32767)
TSPL = HSPL // 128        # 21 tiles in section A

f16d, f32d, i16d = mybir.dt.float16, mybir.dt.float32, mybir.dt.int16
f8d = mybir.dt.float8e4

LAST_EXEC_NS = None
LAST_RESULTS = None
_BUILD_CACHE = {}

def _wrap16(lst):
    """DMA_GATHER index layout: element i at [i%16, i//16], replicated x8."""
    return np.tile(lst.reshape(-1, 16).T, (8, 1))


def _preprocess(row, col):
    order = np.argsort(row, kind="stable")
    row_s = row[order].astype(np.int64)
    col_s = col[order].astype(np.int64)
    counts = np.bincount(row_s // BLK, minlength=NPAD // BLK)
    starts = np.concatenate([[0], np.cumsum(counts)])
    nb = NPAD // BLK
    src_c = col_s // SHARD
    src_r = col_s % SHARD
    in_a = src_r < HSPL
    idx_a = src_c * HSPL + src_r
    idx_b = src_c * (SHARD - HSPL) + (src_r - HSPL)
    nlo = np.zeros(nb, np.int64)
    for b in range(nb):
        nlo[b] = int(in_a[starts[b]:starts[b + 1]].sum())
    nhi = counts - nlo
    cpl = int(((nlo + 127) // 128).max())
    cph = int(((nhi + 127) // 128).max())
    cpe = cpl + cph
    cilo = np.zeros((NC, 128, NBPC * cpl * 8), np.int16)
    cihi = np.zeros((NC, 128, NBPC * cph * 8), np.int16)
    oht = np.zeros((NC, NBPC, 128, cpe * 128), np.float16)
    ohh = np.zeros((NC, NBPC, 128, cpe * 128), np.float16)
    iota = np.arange(128)
    for b in range(nb):
        c, bl = divmod(b, NBPC)
        s, e = starts[b], starts[b + 1]
        rloc = row_s[s:e] - b * BLK
        m = in_a[s:e]
        lo_c, lo_r = idx_a[s:e][m], rloc[m]
        hi_c, hi_r = idx_b[s:e][~m], rloc[~m]
        lst = np.zeros(cpl * 128, np.int16)
        lst[:len(lo_c)] = lo_c
        cilo[c, :, bl * cpl * 8:(bl + 1) * cpl * 8] = _wrap16(lst)
        rlo = np.full(cpl * 128, 200.0, np.float16)
        rlo[:len(lo_r)] = lo_r
        lst2 = np.zeros(cph * 128, np.int16)
        lst2[:len(hi_c)] = hi_c
        cihi[c, :, bl * cph * 8:(bl + 1) * cph * 8] = _wrap16(lst2)
        rhi = np.full(cph * 128, 200.0, np.float16)
        rhi[:len(hi_r)] = hi_r
        rf = np.concatenate([rlo, rhi])
        oht[c, bl] = (iota[:, None] == rf[None, :]).astype(np.float16)
        rfs = rf.reshape(cpe, 128)
        for cc in range(cpe):
            ohh[c, bl][:, cc * 128:(cc + 1) * 128] = (
                rfs[cc][:, None] == iota[None, :]).astype(np.float16)
    return cilo, cihi, oht, ohh, cpl, cph


def _build(cpl, cph):
    key = (cpl, cph, os.environ.get("GAT_FP8_OH", "1"),
           os.environ.get("GAT_PRELU", "1"), os.environ.get("GAT_MAXCH", "16"))
    if key in _BUILD_CACHE:
        return _BUILD_CACHE[key]
    cpe = cpl + cph
    ohd = f8d if int(os.environ.get("GAT_FP8_OH", "1")) else f16d
    use_prelu = bool(int(os.environ.get("GAT_PRELU", "1")))
    nc = bacc.Bacc("TRN2", target_bir_lowering=False, debug=False,
                   enable_asserts=True, num_devices=NC, num_swdge_queues=4)
    xt = nc.dram_tensor("xt", [NBPC, 128, KT * 128], f16d, kind="ExternalInput")
    w1 = nc.dram_tensor("w1", [KT * 128, DW1], f16d, kind="ExternalInput")
    w2 = nc.dram_tensor("w2", [KT * 128, DW2], f16d, kind="ExternalInput")
    cilo = nc.dram_tensor("cilo", [128, NBPC * cpl * 8], i16d, kind="ExternalInput")
    cihi = nc.dram_tensor("cihi", [128, NBPC * cph * 8], i16d, kind="ExternalInput")
    ohtT = nc.dram_tensor("ohtT", [NBPC, 128, cpe * 128], ohd,
                          kind="ExternalInput")
    ohhT = nc.dram_tensor("ohhT", [NBPC, 128, cpe * 128], ohd,
                          kind="ExternalInput")
    out = nc.dram_tensor("out", [SHARD, NCLASS], f32d, kind="ExternalOutput")

    AF, ALU = mybir.ActivationFunctionType, mybir.AluOpType
    HSPB = SHARD - HSPL

    qctr = [0]
    maxch = int(os.environ.get("GAT_MAXCH", "16"))

    def gather_sect(dst, table, idx_t, icol0, nch, elem):
        a = 0
        while a < nch:
            b = min(a + maxch, nch)
            nc.gpsimd.dma_gather(
                out_ap=dst[:, a * elem:b * elem]
                    .rearrange("p (c e) -> p c e", e=elem),
                in_ap=table,
                idxs_ap=idx_t[:, icol0 + a * 8:icol0 + b * 8],
                num_idxs=(b - a) * 128, num_idxs_reg=(b - a) * 128,
                elem_size=elem, queue_num=qctr[0] % 4)
            qctr[0] += 1
            a = b

    with tile.TileContext(nc) as tc:
        with tc.tile_pool(name="res", bufs=1) as res, \
             tc.tile_pool(name="dram", bufs=1, space="DRAM") as drp:
            # shard tables; section-A collective issues mid-phase (a Shared
            # tensor may only be written by ONE instruction, so chunking
            # finer than per-section is not possible)
            tab1sA = drp.tile([HSPL, TW1], f16d)
            tab1sB = drp.tile([HSPB, TW1], f16d)
            tab2sA = drp.tile([HSPL, TW2], f16d)
            tab2sB = drp.tile([HSPB, TW2], f16d)
            tab1a = drp.tile([NC * HSPL, TW1], f16d, addr_space="Shared")
            tab1b = drp.tile([NC * HSPB, TW1], f16d, addr_space="Shared")
            tab2a = drp.tile([NC * HSPL, TW2], f16d, addr_space="Shared")
            tab2b = drp.tile([NC * HSPB, TW2], f16d, addr_space="Shared")

            def ag_sect(src, dst):
                nc.gpsimd.collective_compute(
                    "AllGather", ALU.bypass, replica_groups=[list(range(NC))],
                    ins=[src.opt()],
                    outs=[dst[:].rearrange("(c r) d -> c r d", c=NC)])

            w1_t = res.tile([128, KT * DW1], f16d)
            w2_t = res.tile([128, KT * DW2], f16d)
            for k in range(KT):
                nc.sync.dma_start(out=w1_t[:, k * DW1:(k + 1) * DW1],
                                  in_=w1[k * 128:(k + 1) * 128, :])
                nc.sync.dma_start(out=w2_t[:, k * DW2:(k + 1) * DW2],
                                  in_=w2[k * 128:(k + 1) * 128, :])
            cilo_t = res.tile([128, NBPC * cpl * 8], i16d)
            cihi_t = res.tile([128, NBPC * cph * 8], i16d)
            nc.sync.dma_start(out=cilo_t[:], in_=cilo[:, :])
            nc.sync.dma_start(out=cihi_t[:], in_=cihi[:, :])
            ident = res.tile([128, 128], f16d)
            make_identity(nc, ident[:])
            fd_sb = res.tile([128, NBPC * 8], f16d)
            fd2e_sb = res.tile([128, NBPC * cpe], f32d)

            # ---------------- Phase A ----------------
            with nc.named_scope("phaseA"), \
                 tc.tile_pool(name="pa", bufs=3) as pa, \
                 tc.tile_pool(name="ppa", bufs=2, space="PSUM") as ppa:
                for nt in range(NBPC):
                    psA = ppa.tile([128, 512], f32d, tag="psA")
                    psB = ppa.tile([128, 16], f32d, tag="psB")
                    xk4 = pa.tile([128, KT * 128], f16d, tag="xk4")
                    nc.sync.dma_start(out=xk4[:], in_=xt[nt, :, :])
                    for k in range(KT):
                        nc.tensor.matmul(out=psA[:],
                                         lhsT=xk4[:, k * 128:(k + 1) * 128],
                                         rhs=w1_t[:, k * DW1:k * DW1 + 512],
                                         start=(k == 0), stop=(k == KT - 1))
                        nc.tensor.matmul(out=psB[:],
                                         lhsT=xk4[:, k * 128:(k + 1) * 128],
                                         rhs=w1_t[:, k * DW1 + 512:(k + 1) * DW1],
                                         start=(k == 0), stop=(k == KT - 1))
                    whf = pa.tile([128, DW1], f16d, tag="whf")
                    nc.vector.tensor_copy(out=whf[:, :512], in_=psA[:])
                    nc.scalar.activation(out=whf[:, 512:520], in_=psB[:, 8:16],
                                         func=AF.Copy)
                    nc.vector.tensor_tensor(out=whf[:, 520:528], in0=psB[:, 8:16],
                                            in1=whf[:, 512:520], op=ALU.subtract)
                    nc.vector.tensor_copy(out=fd_sb[:, nt * 8:(nt + 1) * 8],
                                          in_=psB[:, 0:8])
                    if nt < TSPL:
                        r0 = nt * 128
                        nc.sync.dma_start(out=tab1sA[r0:r0 + 128, 0:DW1],
                                          in_=whf[:])
                    else:
                        r0 = (nt - TSPL) * 128
                        nc.sync.dma_start(out=tab1sB[r0:r0 + 128, 0:DW1],
                                          in_=whf[:])
                    if nt == TSPL - 1:
                        with nc.named_scope("ag1"):
                            ag_sect(tab1sA, tab1a)
                    if nt == NBPC - 1:
                        with nc.named_scope("ag1"):
                            ag_sect(tab1sB, tab1b)

            # ---------------- Phase B ----------------
            with nc.named_scope("phaseB"), \
                 tc.tile_pool(name="pb", bufs=2) as pb, \
                 tc.tile_pool(name="pg", bufs=3) as pg, \
                 tc.tile_pool(name="ppb", bufs=2, space="PSUM") as ppb, \
                 tc.tile_pool(name="ppf", bufs=2, space="PSUM") as ppf, \
                 tc.tile_pool(name="ppt", bufs=2, space="PSUM") as ppt:
                def issue_b(bl):
                    G = pg.tile([128, cpe * TW1], f16d, tag="G")
                    OH = pg.tile([128, cpe * 128], ohd, tag="OH")
                    OHT = pg.tile([128, cpe * 128], ohd, tag="OHT")
                    nc.sync.dma_start(out=OHT[:], in_=ohtT[bl, :, :])
                    nc.sync.dma_start(out=OH[:], in_=ohhT[bl, :, :])
                    gather_sect(G[:, :cpl * TW1], tab1a[:, :],
                                cilo_t, bl * cpl * 8, cpl, TW1)
                    gather_sect(G[:, cpl * TW1:], tab1b[:, :],
                                cihi_t, bl * cph * 8, cph, TW1)
                    return G, OH, OHT

                pend = [issue_b(0), issue_b(1)]
                for bl in range(NBPC):
                    pnum = ppb.tile([128, 512], f32d, tag="pnum")
                    aux = ppf.tile([128, 8 + cpe * 9 + DW2], f32d, tag="aux")
                    pden = aux[:, 0:8]
                    pfd = aux[:, 8:8 + cpe * 8]
                    pfd2 = aux[:, 8 + cpe * 8:8 + cpe * 9]
                    ps2 = aux[:, 8 + cpe * 9:8 + cpe * 9 + DW2]
                    G, OH, OHT = pend.pop(0)
                    if bl + 2 < NBPC:
                        pend.append(issue_b(bl + 2))
                    for c in range(cpe):
                        nc.tensor.matmul(out=pfd[:, c * 8:(c + 1) * 8],
                                         lhsT=OHT[:, c * 128:(c + 1) * 128],
                                         rhs=fd_sb[:, bl * 8:(bl + 1) * 8],
                                         start=True, stop=True)
                    e1 = pb.tile([128, cpe * 8], f32d, tag="e1")
                    lr = pb.tile([128, cpe * 8], f32d, tag="lr")
                    p16 = pb.tile([128, cpe * 8], f16d, tag="p16")
                    Gr = G[:].rearrange("p (c d) -> p c d", d=TW1)
                    nc.vector.tensor_tensor(
                        out=e1[:].rearrange("p (c f) -> p c f", c=cpe),
                        in0=Gr[:, :, 512:520],
                        in1=Gr[:, :, 520:528], op=ALU.add)
                    nc.vector.tensor_tensor(out=e1[:], in0=e1[:], in1=pfd,
                                            op=ALU.add)
                    if use_prelu:
                        nc.scalar.activation(out=lr[:], in_=e1[:],
                                             func=AF.Prelu, alpha=ALPHA)
                    else:
                        nc.vector.tensor_scalar_mul(lr[:], e1[:], ALPHA)
                        nc.vector.tensor_tensor(out=lr[:], in0=e1[:],
                                                in1=lr[:], op=ALU.max)
                    nc.scalar.activation(out=p16[:], in_=lr[:], func=AF.Exp)
                    # R[p, c, f, h] = Wh[p, c, f, h] * p16[p, c, h] -- the
                    # h-minor layout keeps innermost stride 1 => DVE 2x mode
                    R = pb.tile([128, cpe * 512], f16d, tag="R")
                    nc.vector.tensor_tensor(
                        out=R[:].rearrange("p (c f h) -> p c f h", f=64, h=8),
                        in0=Gr[:, :, 0:512]
                            .rearrange("p c (f h) -> p c f h", f=64),
                        in1=p16[:].rearrange("p (c o h) -> p c o h", o=1, h=8)
                            .to_broadcast([128, cpe, 64, 8]),
                        op=ALU.mult)
                    for i in range(cpe):
                        nc.tensor.matmul(out=pnum[:],
                                         lhsT=OH[:, i * 128:(i + 1) * 128],
                                         rhs=R[:, i * 512:(i + 1) * 512],
                                         start=(i == 0), stop=(i == cpe - 1))
                        nc.tensor.matmul(out=pden,
                                         lhsT=OH[:, i * 128:(i + 1) * 128],
                                         rhs=p16[:, i * 8:(i + 1) * 8],
                                         start=(i == 0), stop=(i == cpe - 1))
                    dcl = pb.tile([128, 8], f32d, tag="dcl")
                    nc.vector.tensor_scalar_max(dcl[:], pden, 1e-30)
                    nc.vector.reciprocal(out=dcl[:], in_=dcl[:])
                    ha = pb.tile([128, 512], f16d, tag="ha")
                    nc.vector.tensor_tensor(
                        out=ha[:].rearrange("p (f h) -> p f h", f=64),
                        in0=pnum[:].rearrange("p (f h) -> p f h", f=64),
                        in1=dcl[:].rearrange("p (o h) -> p o h", o=1)
                            .to_broadcast([128, 64, 8]),
                        op=ALU.mult)
                    # elu: h16 = max(ha, min(e^ha, 1) - 1)  (exp monotone)
                    hm = pb.tile([128, 512], f16d, tag="hm")
                    nc.scalar.activation(out=hm[:], in_=ha[:], func=AF.Exp)
                    nc.vector.tensor_scalar(out=hm[:], in0=hm[:],
                                            scalar1=1.0, scalar2=-1.0,
                                            op0=ALU.min, op1=ALU.add)
                    h16 = pb.tile([128, 512], f16d, tag="h16")
                    nc.vector.tensor_tensor(out=h16[:], in0=hm[:], in1=ha[:],
                                            op=ALU.max)
                    for k in range(KT):
                        pt = ppt.tile([128, 128], f16d, tag="pt")
                        nc.tensor.transpose(out=pt[:],
                                            in_=h16[:, k * 128:(k + 1) * 128],
                                            identity=ident[:])
                        ht = pb.tile([128, 128], f16d, tag="ht")
                        if k % 2 == 0:
                            nc.vector.tensor_copy(out=ht[:], in_=pt[:])
                        else:
                            nc.scalar.activation(out=ht[:], in_=pt[:],
                                                 func=AF.Copy)
                        nc.tensor.matmul(out=ps2, lhsT=ht[:],
                                         rhs=w2_t[:, k * DW2:(k + 1) * DW2],
                                         start=(k == 0), stop=(k == KT - 1))
                    t2 = pb.tile([128, DW2], f16d, tag="t2")
                    nc.scalar.activation(out=t2[:, 0:64], in_=ps2[:, 0:64],
                                         func=AF.Copy)
                    nc.scalar.activation(out=t2[:, 64:65], in_=ps2[:, 65:66],
                                         func=AF.Copy)
                    nc.vector.tensor_tensor(out=t2[:, 65:66], in0=ps2[:, 65:66],
                                            in1=t2[:, 64:65], op=ALU.subtract)
                    fd2 = pb.tile([128, 1], f16d, tag="fd2")
                    nc.scalar.activation(out=fd2[:], in_=ps2[:, 64:65],
                                         func=AF.Copy)
                    for c in range(cpe):
                        nc.tensor.matmul(out=pfd2[:, c:c + 1],
                                         lhsT=OHT[:, c * 128:(c + 1) * 128],
                                         rhs=fd2[:],
                                         start=True, stop=True)
                    nc.vector.tensor_copy(
                        out=fd2e_sb[:, bl * cpe:(bl + 1) * cpe], in_=pfd2)
                    if bl < TSPL:
                        r0 = bl * 128
                        nc.sync.dma_start(out=tab2sA[r0:r0 + 128, 0:DW2],
                                          in_=t2[:])
                    else:
                        r0 = (bl - TSPL) * 128
                        nc.sync.dma_start(out=tab2sB[r0:r0 + 128, 0:DW2],
                                          in_=t2[:])
                    if bl == TSPL - 1:
                        with nc.named_scope("ag2"):
                            ag_sect(tab2sA, tab2a)
                    if bl == NBPC - 1:
                        with nc.named_scope("ag2"):
                            ag_sect(tab2sB, tab2b)

            # ---------------- Phase C ----------------
            with nc.named_scope("phaseC"), \
                 tc.tile_pool(name="pc", bufs=3) as pc, \
                 tc.tile_pool(name="pg2", bufs=3) as pg2, \
                 tc.tile_pool(name="ppc", bufs=2, space="PSUM") as ppc:
                def issue_c(bl):
                    G2 = pg2.tile([128, cpe * TW2], f16d, tag="G2")
                    OH2 = pg2.tile([128, cpe * 128], ohd, tag="OH2")
                    nc.sync.dma_start(out=OH2[:], in_=ohhT[bl, :, :])
                    gather_sect(G2[:, :cpl * TW2], tab2a[:, :],
                                cilo_t, bl * cpl * 8, cpl, TW2)
                    gather_sect(G2[:, cpl * TW2:], tab2b[:, :],
                                cihi_t, bl * cph * 8, cph, TW2)
                    return G2, OH2

                pend2 = [issue_c(0), issue_c(1)]
                for bl in range(NBPC):
                    rows = slice(bl * 128, (bl + 1) * 128)
                    ps3 = ppc.tile([128, 65], f32d, tag="ps3")
                    G2, OH2 = pend2.pop(0)
                    if bl + 2 < NBPC:
                        pend2.append(issue_c(bl + 2))
                    e2 = pc.tile([128, cpe], f32d, tag="e2")
                    lr2 = pc.tile([128, cpe], f32d, tag="lr2")
                    p2 = pc.tile([128, cpe], f16d, tag="p2")
                    G2r = G2[:].rearrange("p (c d) -> p c d", d=TW2)
                    nc.vector.tensor_tensor(
                        out=e2[:].rearrange("p (c o) -> p c o", o=1),
                        in0=G2r[:, :, 64:65],
                        in1=G2r[:, :, 65:66], op=ALU.add)
                    nc.vector.tensor_tensor(
                        out=e2[:], in0=e2[:],
                        in1=fd2e_sb[:, bl * cpe:(bl + 1) * cpe], op=ALU.add)
                    if use_prelu:
                        nc.scalar.activation(out=lr2[:], in_=e2[:],
                                             func=AF.Prelu, alpha=ALPHA)
                    else:
                        nc.vector.tensor_scalar_mul(lr2[:], e2[:], ALPHA)
                        nc.vector.tensor_tensor(out=lr2[:], in0=e2[:],
                                                in1=lr2[:], op=ALU.max)
                    nc.scalar.activation(out=p2[:], in_=lr2[:], func=AF.Exp)
                    R2 = pc.tile([128, cpe * 65], f16d, tag="R2")
                    R2r = R2[:].rearrange("p (c d) -> p c d", d=65)
                    nc.vector.tensor_tensor(
                        out=R2r[:, :, 0:64],
                        in0=G2r[:, :, 0:64],
                        in1=p2[:].to_broadcast([128, cpe, 64]),
                        op=ALU.mult)
                    nc.vector.tensor_copy(
                        out=R2r[:, :, 64:65],
                        in_=p2[:].rearrange("p (c o) -> p c o", o=1))
                    for i in range(cpe):
                        nc.tensor.matmul(out=ps3[:],
                                         lhsT=OH2[:, i * 128:(i + 1) * 128],
                                         rhs=R2[:, i * 65:(i + 1) * 65],
                                         start=(i == 0), stop=(i == cpe - 1))
                    d2c = pc.tile([128, 1], f32d, tag="d2c")
                    nc.vector.tensor_scalar_max(d2c[:], ps3[:, 64:65], 1e-30)
                    nc.vector.reciprocal(out=d2c[:], in_=d2c[:])
                    o = pc.tile([128, 64], f32d, tag="o")
                    nc.vector.tensor_tensor(
                        out=o[:].rearrange("p (c f) -> p c f", c=1),
                        in0=ps3[:, 0:64].rearrange("p (c f) -> p c f", c=1),
                        in1=d2c[:].to_broadcast([128, 1, 64]),
                        op=ALU.mult)
                    nc.sync.dma_start(out=out[rows, :], in_=o[:])

    nc.compile()
    _BUILD_CACHE[key] = nc
    return nc


def kernel(**inputs):
    global LAST_EXEC_NS, LAST_RESULTS
    x = inputs["x"].astype(np.float32)
    row = inputs["row"].astype(np.int64)
    col = inputs["col"].astype(np.int64)
    W, a = inputs["W"].astype(np.float32), inputs["a"].astype(np.float32)
    W_out = inputs["W_out"].astype(np.float32)
    a_out = inputs["a_out"].astype(np.float32)

    cilo, cihi, oht, ohh, cpl, cph = _preprocess(row, col)

    # head-MINOR (f,h) feature order for layer-1 Wh and layer-2 rows
    W_cat = np.stack([W[h] for h in range(NHEADS)], axis=-1)  # [in, f, h]
    W_cat = W_cat.reshape(NFEAT, NHID * NHEADS)
    WA_dst = np.stack([W[h] @ a[h, :NHID] for h in range(NHEADS)], 1)
    WA_src = np.stack([W[h] @ a[h, NHID:] for h in range(NHEADS)], 1)
    w1_np = np.concatenate([W_cat, WA_dst, WA_src], 1).astype(np.float16)
    w2full = np.concatenate([W_out, (W_out @ a_out[:NCLASS])[:, None],
                             (W_out @ a_out[NCLASS:])[:, None]], 1)
    idx = np.arange(NHID * NHEADS)
    perm = (idx % NHEADS) * NHID + idx // NHEADS   # (f,h) -> h*64+f
    w2_np = w2full[perm, :].astype(np.float16)

    x_pad = np.zeros((NPAD, NFEAT), np.float16)
    x_pad[:N] = x

    nc = _build(cpl, cph)

    fp8 = bool(int(os.environ.get("GAT_FP8_OH", "1")))
    ohdt = mybir.dt.np(f8d) if fp8 else np.float16
    in_maps = []
    for c in range(NC):
        xs = x_pad[c * SHARD:(c + 1) * SHARD]            # [6272, 512]
        xt = (xs.reshape(NBPC, 128, KT, 128)             # [nt, n, k, f]
                .transpose(0, 3, 2, 1)                   # [nt, f, k, n]
                .reshape(NBPC, 128, KT * 128)).copy()
        in_maps.append({"xt": xt, "w1": w1_np, "w2": w2_np,
                        "cilo": cilo[c], "cihi": cihi[c],
                        "ohtT": oht[c].astype(ohdt),
                        "ohhT": ohh[c].astype(ohdt)})

    trace = bool(int(os.environ.get("GAT_TRACE", "0")))
    res = run_bass_kernel_spmd(nc, in_maps, list(range(NC)), trace=trace,
                               trace_cores=list(range(NC)) if trace else None)
    LAST_EXEC_NS = res.exec_time_ns
    LAST_RESULTS = res
    outs = [res.results[c]["out"] for c in range(NC)]
    return np.concatenate(outs, 0)[:N].astype(np.float32)


# revision 16
# speedup vs baseline: 14191.6611x; 14191.6611x over previous
"""GAT (2-layer, 8-head) Trainium2 kernel over 8 NeuronCores.

Strategy (edge-cut node sharding), v2:
- Pad N 50000->50176 = 8 shards * 6272. Core c owns nodes [6272c, 6272(c+1)).
- Host: sort edges by dest, bucket into 128-node blocks. Within a block, edges
  are split lo (shard-row < HSPL) / hi, because the batched DMA_GATHER takes
  int16 indices (<=32767) and the full table has 50176 rows. HSPL=2688 is
  128-aligned so the A/B table sections align to 128-row tiles.
- Table-1 row: [Wh in (f,h) head-MINOR order 512 | f_src f16 8 | f_src
  residual 8 | pad -> 640] (1280B, elem %256B). Head-minor order makes the
  per-edge attention scale R = p (*) Wh a DVE TENSOR_TENSOR with an OUTER-dim
  broadcast and innermost stride 1 -> 2x_1P perf mode (the head-major layout
  forces a stride-0 innermost broadcast = 1x).
- One dma_gather per (block, section) (fixed SWDGE overhead ~1us dominates,
  so fewer+bigger gathers), round-robin over 4 queues.
- One-hots (OH: [edge,dst], OHT: [dst,edge]) shipped as fp8e4 (0/1 exact):
  halves their HBM traffic and speeds LDWEIGHTS via FWL.
- leaky-relu on the Scalar engine via Prelu(alpha) (same act table set as Exp
  and Copy -> no table reloads); ELU via hm=Exp(ha) then min/add tensor_scalar
  + max (exp is monotone: e^min(x,0) == min(e^x, 1)).
- AllGathers chunked into 3 pieces per table section and interleaved into the
  producing phase so the collective overlaps compute.
- Softmax needs no segment-max: logits are O(6) so exp never overflows, and
  normalization commutes with the scatter-sum (divide once per node).
"""
import os
import sys
sys.path.insert(0, "/opt/trn_rl_repo")
import numpy as np

import concourse.tile as tile
from concourse import bass, bacc, mybir
from concourse.bass_utils import run_bass_kernel_spmd
from concourse.masks import make_identity

N, E = 50000, 800000
NFEAT, NHID, NHEADS, NCLASS = 512, 64, 8, 64
ALPHA = 0.2
NC = 8
NPAD = 50176
SHARD = NPAD // NC        # 6272
BLK = 128
NBPC = SHARD // BLK       # 49 blocks per core
KT = NFEAT // 128         # 4 k-tiles
DW1 = NFEAT + 16          # 528: Wh | src | src_residual
TW1 = 640                 # table-1 row (1280B, %256B for dma_gather)
DW2 = NCLASS + 2          # 66:  Wh2 | src | src_residual
TW2 = 128                 # table-2 row (256B)
HSPL = 2688               # 21*128; section A rows per shard (8*2688<=32767)
TSPL = HSPL // 128        # 21 tiles in section A

f16d, f32d, i16d = mybir.dt.float16, mybir.dt.float32, mybir.dt.int16
f8d = mybir.dt.float8e4

LAST_EXEC_NS = None
LAST_RESULTS = None
_BUILD_CACHE = {}

def _wrap16(lst):
    """DMA_GATHER index layout: element i at [i%16, i//16], replicated x8."""
    return np.tile(lst.reshape(-1, 16).T, (8, 1))


def _preprocess(row, col):
    order = np.argsort(row, kind="stable")
    row_s = row[order].astype(np.int64)
    col_s = col[order].astype(np.int64)
    counts = np.bincount(row_s // BLK, minlength=NPAD // BLK)
    starts = np.concatenate([[0], np.cumsum(counts)])
    nb = NPAD // BLK
    src_c = col_s // SHARD
    src_r = col_s % SHARD
    in_a = src_r < HSPL
    idx_a = src_c * HSPL + src_r
    idx_b = src_c * (SHARD - HSPL) + (src_r - HSPL)
    nlo = np.zeros(nb, np.int64)
    for b in range(nb):
        nlo[b] = int(in_a[starts[b]:starts[b + 1]].sum())
    nhi = counts - nlo
    cpl = int(((nlo + 127) // 128).max())
    cph = int(((nhi + 127) // 128).max())
    cpe = cpl + cph
    cilo = np.zeros((NC, 128, NBPC * cpl * 8), np.int16)
    cihi = np.zeros((NC, 128, NBPC * cph * 8), np.int16)
    oht = np.zeros((NC, NBPC, 128, cpe * 128), np.float16)
    ohh = np.zeros((NC, NBPC, 128, cpe * 128), np.float16)
    iota = np.arange(128)
    for b in range(nb):
        c, bl = divmod(b, NBPC)
        s, e = starts[b], starts[b + 1]
        rloc = row_s[s:e] - b * BLK
        m = in_a[s:e]
        lo_c, lo_r = idx_a[s:e][m], rloc[m]
        hi_c, hi_r = idx_b[s:e][~m], rloc[~m]
        lst = np.zeros(cpl * 128, np.int16)
        lst[:len(lo_c)] = lo_c
        cilo[c, :, bl * cpl * 8:(bl + 1) * cpl * 8] = _wrap16(lst)
        rlo = np.full(cpl * 128, 200.0, np.float16)
        rlo[:len(lo_r)] = lo_r
        lst2 = np.zeros(cph * 128, np.int16)
        lst2[:len(hi_c)] = hi_c
        cihi[c, :, bl * cph * 8:(bl + 1) * cph * 8] = _wrap16(lst2)
        rhi = np.full(cph * 128, 200.0, np.float16)
        rhi[:len(hi_r)] = hi_r
        rf = np.concatenate([rlo, rhi])
        oht[c, bl] = (iota[:, None] == rf[None, :]).astype(np.float16)
        rfs = rf.reshape(cpe, 128)
        for cc in range(cpe):
            ohh[c, bl][:, cc * 128:(cc + 1) * 128] = (
                rfs[cc][:, None] == iota[None, :]).astype(np.float16)
    return cilo, cihi, oht, ohh, cpl, cph


def _build(cpl, cph):
    key = (cpl, cph, os.environ.get("GAT_FP8_OH", "1"),
           os.environ.get("GAT_PRELU", "1"), os.environ.get("GAT_MAXCH", "16"))
    if key in _BUILD_CACHE:
        return _BUILD_CACHE[key]
    cpe = cpl + cph
    ohd = f8d if int(os.environ.get("GAT_FP8_OH", "1")) else f16d
    use_prelu = bool(int(os.environ.get("GAT_PRELU", "1")))
    nc = bacc.Bacc("TRN2", target_bir_lowering=False, debug=False,
                   enable_asserts=True, num_devices=NC, num_swdge_queues=4)
    xt = nc.dram_tensor("xt", [NBPC, 128, KT * 128], f16d, kind="ExternalInput")
    w1 = nc.dram_tensor("w1", [KT * 128, DW1], f16d, kind="ExternalInput")
    w2 = nc.dram_tensor("w2", [KT * 128, DW2], f16d, kind="ExternalInput")
    cilo = nc.dram_tensor("cilo", [128, NBPC * cpl * 8], i16d, kind="ExternalInput")
    cihi = nc.dram_tensor("cihi", [128, NBPC * cph * 8], i16d, kind="ExternalInput")
    ohtT = nc.dram_tensor("ohtT", [NBPC, 128, cpe * 128], ohd,
                          kind="ExternalInput")
    ohhT = nc.dram_tensor("ohhT", [NBPC, 128, cpe * 128], ohd,
                          kind="ExternalInput")
    out = nc.dram_tensor("out", [SHARD, NCLASS], f32d, kind="ExternalOutput")

    AF, ALU = mybir.ActivationFunctionType, mybir.AluOpType
    HSPB = SHARD - HSPL

    qctr = [0]
    # SWDGE dma_gather hangs above 512 idxs/instruction -> at most 4 chunks
    maxch = int(os.environ.get("GAT_MAXCH", "4"))

    def gather_sect(dst, table, idx_t, icol0, nch, elem):
        a = 0
        while a < nch:
            b = min(a + maxch, nch)
            nc.gpsimd.dma_gather(
                out_ap=dst[:, a * elem:b * elem]
                    .rearrange("p (c e) -> p c e", e=elem),
                in_ap=table,
                idxs_ap=idx_t[:, icol0 + a * 8:icol0 + b * 8],
                num_idxs=(b - a) * 128, num_idxs_reg=(b - a) * 128,
                elem_size=elem, queue_num=qctr[0] % 4)
            qctr[0] += 1
            a = b

    with tile.TileContext(nc) as tc:
        with tc.tile_pool(name="res", bufs=1) as res, \
             tc.tile_pool(name="dram", bufs=1, space="DRAM") as drp:
            # shard tables; section-A collective issues mid-phase (a Shared
            # tensor may only be written by ONE instruction, so chunking
            # finer than per-section is not possible)
            tab1sA = drp.tile([HSPL, TW1], f16d)
            tab1sB = drp.tile([HSPB, TW1], f16d)
            tab2sA = drp.tile([HSPL, TW2], f16d)
            tab2sB = drp.tile([HSPB, TW2], f16d)
            tab1a = drp.tile([NC * HSPL, TW1], f16d, addr_space="Shared")
            tab1b = drp.tile([NC * HSPB, TW1], f16d, addr_space="Shared")
            tab2a = drp.tile([NC * HSPL, TW2], f16d, addr_space="Shared")
            tab2b = drp.tile([NC * HSPB, TW2], f16d, addr_space="Shared")

            def ag_sect(src, dst):
                nc.gpsimd.collective_compute(
                    "AllGather", ALU.bypass, replica_groups=[list(range(NC))],
                    ins=[src.opt()],
                    outs=[dst[:].rearrange("(c r) d -> c r d", c=NC)])

            w1_t = res.tile([128, KT * DW1], f16d)
            w2_t = res.tile([128, KT * DW2], f16d)
            for k in range(KT):
                nc.sync.dma_start(out=w1_t[:, k * DW1:(k + 1) * DW1],
                                  in_=w1[k * 128:(k + 1) * 128, :])
                nc.sync.dma_start(out=w2_t[:, k * DW2:(k + 1) * DW2],
                                  in_=w2[k * 128:(k + 1) * 128, :])
            cilo_t = res.tile([128, NBPC * cpl * 8], i16d)
            cihi_t = res.tile([128, NBPC * cph * 8], i16d)
            nc.sync.dma_start(out=cilo_t[:], in_=cilo[:, :])
            nc.sync.dma_start(out=cihi_t[:], in_=cihi[:, :])
            ident = res.tile([128, 128], f16d)
            make_identity(nc, ident[:])
            fd_sb = res.tile([128, NBPC * 8], f16d)
            fd2e_sb = res.tile([128, NBPC * cpe], f32d)

            # ---------------- Phase A ----------------
            with nc.named_scope("phaseA"), \
                 tc.tile_pool(name="pa", bufs=3) as pa, \
                 tc.tile_pool(name="ppa", bufs=2, space="PSUM") as ppa:
                for nt in range(NBPC):
                    psA = ppa.tile([128, 512], f32d, tag="psA")
                    psB = ppa.tile([128, 16], f32d, tag="psB")
                    xk4 = pa.tile([128, KT * 128], f16d, tag="xk4")
                    nc.sync.dma_start(out=xk4[:], in_=xt[nt, :, :])
                    for k in range(KT):
                        nc.tensor.matmul(out=psA[:],
                                         lhsT=xk4[:, k * 128:(k + 1) * 128],
                                         rhs=w1_t[:, k * DW1:k * DW1 + 512],
                                         start=(k == 0), stop=(k == KT - 1))
                        nc.tensor.matmul(out=psB[:],
                                         lhsT=xk4[:, k * 128:(k + 1) * 128],
                                         rhs=w1_t[:, k * DW1 + 512:(k + 1) * DW1],
                                         start=(k == 0), stop=(k == KT - 1))
                    whf = pa.tile([128, DW1], f16d, tag="whf")
                    nc.vector.tensor_copy(out=whf[:, :512], in_=psA[:])
                    nc.scalar.activation(out=whf[:, 512:520], in_=psB[:, 8:16],
                                         func=AF.Copy)
                    nc.vector.tensor_tensor(out=whf[:, 520:528], in0=psB[:, 8:16],
                                            in1=whf[:, 512:520], op=ALU.subtract)
                    nc.vector.tensor_copy(out=fd_sb[:, nt * 8:(nt + 1) * 8],
                                          in_=psB[:, 0:8])
                    if nt < TSPL:
                        r0 = nt * 128
                        nc.sync.dma_start(out=tab1sA[r0:r0 + 128, 0:DW1],
                                          in_=whf[:])
                    else:
                        r0 = (nt - TSPL) * 128
                        nc.sync.dma_start(out=tab1sB[r0:r0 + 128, 0:DW1],
                                          in_=whf[:])
                    if nt == TSPL - 1:
                        with nc.named_scope("ag1"):
                            ag_sect(tab1sA, tab1a)
                    if nt == NBPC - 1:
                        with nc.named_scope("ag1"):
                            ag_sect(tab1sB, tab1b)

            # ---------------- Phase B ----------------
            with nc.named_scope("phaseB"), \
                 tc.tile_pool(name="pb", bufs=2) as pb, \
                 tc.tile_pool(name="pg", bufs=3) as pg, \
                 tc.tile_pool(name="ppb", bufs=2, space="PSUM") as ppb, \
                 tc.tile_pool(name="ppf", bufs=2, space="PSUM") as ppf, \
                 tc.tile_pool(name="ppt", bufs=2, space="PSUM") as ppt:
                def issue_b(bl):
                    G = pg.tile([128, cpe * TW1], f16d, tag="G")
                    OH = pg.tile([128, cpe * 128], ohd, tag="OH")
                    OHT = pg.tile([128, cpe * 128], ohd, tag="OHT")
                    nc.sync.dma_start(out=OHT[:], in_=ohtT[bl, :, :])
                    nc.sync.dma_start(out=OH[:], in_=ohhT[bl, :, :])
                    gather_sect(G[:, :cpl * TW1], tab1a[:, :],
                                cilo_t, bl * cpl * 8, cpl, TW1)
                    gather_sect(G[:, cpl * TW1:], tab1b[:, :],
                                cihi_t, bl * cph * 8, cph, TW1)
                    return G, OH, OHT

                pend = [issue_b(0), issue_b(1)]
                for bl in range(NBPC):
                    pnum = ppb.tile([128, 512], f32d, tag="pnum")
                    aux = ppf.tile([128, 8 + cpe * 9 + DW2], f32d, tag="aux")
                    pden = aux[:, 0:8]
                    pfd = aux[:, 8:8 + cpe * 8]
                    pfd2 = aux[:, 8 + cpe * 8:8 + cpe * 9]
                    ps2 = aux[:, 8 + cpe * 9:8 + cpe * 9 + DW2]
                    G, OH, OHT = pend.pop(0)
                    if bl + 2 < NBPC:
                        pend.append(issue_b(bl + 2))
                    for c in range(cpe):
                        nc.tensor.matmul(out=pfd[:, c * 8:(c + 1) * 8],
                                         lhsT=OHT[:, c * 128:(c + 1) * 128],
                                         rhs=fd_sb[:, bl * 8:(bl + 1) * 8],
                                         start=True, stop=True)
                    e1 = pb.tile([128, cpe * 8], f32d, tag="e1")
                    lr = pb.tile([128, cpe * 8], f32d, tag="lr")
                    p16 = pb.tile([128, cpe * 8], f16d, tag="p16")
                    Gr = G[:].rearrange("p (c d) -> p c d", d=TW1)
                    nc.vector.tensor_tensor(
                        out=e1[:].rearrange("p (c f) -> p c f", c=cpe),
                        in0=Gr[:, :, 512:520],
                        in1=Gr[:, :, 520:528], op=ALU.add)
                    nc.vector.tensor_tensor(out=e1[:], in0=e1[:], in1=pfd,
                                            op=ALU.add)
                    if use_prelu:
                        nc.scalar.activation(out=lr[:], in_=e1[:],
                                             func=AF.Prelu, alpha=ALPHA)
                    else:
                        nc.vector.tensor_scalar_mul(lr[:], e1[:], ALPHA)
                        nc.vector.tensor_tensor(out=lr[:], in0=e1[:],
                                                in1=lr[:], op=ALU.max)
                    nc.scalar.activation(out=p16[:], in_=lr[:], func=AF.Exp)
                    # R[p, c, f, h] = Wh[p, c, f, h] * p16[p, c, h] -- the
                    # h-minor layout keeps innermost stride 1 => DVE 2x mode
                    R = pb.tile([128, cpe * 512], f16d, tag="R")
                    nc.vector.tensor_tensor(
                        out=R[:].rearrange("p (c f h) -> p c f h", f=64, h=8),
                        in0=Gr[:, :, 0:512]
                            .rearrange("p c (f h) -> p c f h", f=64),
                        in1=p16[:].rearrange("p (c o h) -> p c o h", o=1, h=8)
                            .to_broadcast([128, cpe, 64, 8]),
                        op=ALU.mult)
                    for i in range(cpe):
                        nc.tensor.matmul(out=pnum[:],
                                         lhsT=OH[:, i * 128:(i + 1) * 128],
                                         rhs=R[:, i * 512:(i + 1) * 512],
                                         start=(i == 0), stop=(i == cpe - 1))
                        nc.tensor.matmul(out=pden,
                                         lhsT=OH[:, i * 128:(i + 1) * 128],
                                         rhs=p16[:, i * 8:(i + 1) * 8],
                                         start=(i == 0), stop=(i == cpe - 1))
                    dcl = pb.tile([128, 8], f32d, tag="dcl")
                    nc.vector.tensor_scalar_max(dcl[:], pden, 1e-30)
                    nc.vector.reciprocal(out=dcl[:], in_=dcl[:])
                    ha = pb.tile([128, 512], f16d, tag="ha")
                    nc.vector.tensor_tensor(
                        out=ha[:].rearrange("p (f h) -> p f h", f=64),
                        in0=pnum[:].rearrange("p (f h) -> p f h", f=64),
                        in1=dcl[:].rearrange("p (o h) -> p o h", o=1)
                            .to_broadcast([128, 64, 8]),
                        op=ALU.mult)
                    # elu: h16 = max(ha, min(e^ha, 1) - 1)  (exp monotone)
                    hm = pb.tile([128, 512], f16d, tag="hm")
                    nc.scalar.activation(out=hm[:], in_=ha[:], func=AF.Exp)
                    nc.vector.tensor_scalar(out=hm[:], in0=hm[:],
                                            scalar1=1.0, scalar2=-1.0,
                                            op0=ALU.min, op1=ALU.add)
                    h16 = pb.tile([128, 512], f16d, tag="h16")
                    nc.vector.tensor_tensor(out=h16[:], in0=hm[:], in1=ha[:],
                                            op=ALU.max)
                    for k in range(KT):
                        pt = ppt.tile([128, 128], f16d, tag="pt")
                        nc.tensor.transpose(out=pt[:],
                                            in_=h16[:, k * 128:(k + 1) * 128],
                                            identity=ident[:])
                        ht = pb.tile([128, 128], f16d, tag="ht")
                        if k % 2 == 0:
                            nc.vector.tensor_copy(out=ht[:], in_=pt[:])
                        else:
                            nc.scalar.activation(out=ht[:], in_=pt[:],
                                                 func=AF.Copy)
                        nc.tensor.matmul(out=ps2, lhsT=ht[:],
                                         rhs=w2_t[:, k * DW2:(k + 1) * DW2],
                                         start=(k == 0), stop=(k == KT - 1))
                    t2 = pb.tile([128, DW2], f16d, tag="t2")
                    nc.scalar.activation(out=t2[:, 0:64], in_=ps2[:, 0:64],
                                         func=AF.Copy)
                    nc.scalar.activation(out=t2[:, 64:65], in_=ps2[:, 65:66],
                                         func=AF.Copy)
                    nc.vector.tensor_tensor(out=t2[:, 65:66], in0=ps2[:, 65:66],
                                            in1=t2[:, 64:65], op=ALU.subtract)
                    fd2 = pb.tile([128, 1], f16d, tag="fd2")
                    nc.scalar.activation(out=fd2[:], in_=ps2[:, 64:65],
                                         func=AF.Copy)
                    for c in range(cpe):
                        nc.tensor.matmul(out=pfd2[:, c:c + 1],
                                         lhsT=OHT[:, c * 128:(c + 1) * 128],
                                         rhs=fd2[:],
                                         start=True, stop=True)
                    nc.vector.tensor_copy(
                        out=fd2e_sb[:, bl * cpe:(bl + 1) * cpe], in_=pfd2)
                    if bl < TSPL:
                        r0 = bl * 128
                        nc.sync.dma_start(out=tab2sA[r0:r0 + 128, 0:DW2],
                                          in_=t2[:])
                    else:
                        r0 = (bl - TSPL) * 128
                        nc.sync.dma_start(out=tab2sB[r0:r0 + 128, 0:DW2],
                                          in_=t2[:])
                    if bl == TSPL - 1:
                        with nc.named_scope("ag2"):
                            ag_sect(tab2sA, tab2a)
                    if bl == NBPC - 1:
                        with nc.named_scope("ag2"):
                            ag_sect(tab2sB, tab2b)

            # ---------------- Phase C ----------------
            with nc.named_scope("phaseC"), \
                 tc.tile_pool(name="pc", bufs=3) as pc, \
                 tc.tile_pool(name="pg2", bufs=3) as pg2, \
                 tc.tile_pool(name="ppc", bufs=2, space="PSUM") as ppc:
                def issue_c(bl):
                    G2 = pg2.tile([128, cpe * TW2], f16d, tag="G2")
                    OH2 = pg2.tile([128, cpe * 128], ohd, tag="OH2")
                    nc.sync.dma_start(out=OH2[:], in_=ohhT[bl, :, :])
                    gather_sect(G2[:, :cpl * TW2], tab2a[:, :],
                                cilo_t, bl * cpl * 8, cpl, TW2)
                    gather_sect(G2[:, cpl * TW2:], tab2b[:, :],
                                cihi_t, bl * cph * 8, cph, TW2)
                    return G2, OH2

                pend2 = [issue_c(0), issue_c(1)]
                for bl in range(NBPC):
                    rows = slice(bl * 128, (bl + 1) * 128)
                    ps3 = ppc.tile([128, 65], f32d, tag="ps3")
                    G2, OH2 = pend2.pop(0)
                    if bl + 2 < NBPC:
                        pend2.append(issue_c(bl + 2))
                    e2 = pc.tile([128, cpe], f32d, tag="e2")
                    lr2 = pc.tile([128, cpe], f32d, tag="lr2")
                    p2 = pc.tile([128, cpe], f16d, tag="p2")
                    G2r = G2[:].rearrange("p (c d) -> p c d", d=TW2)
                    nc.vector.tensor_tensor(
                        out=e2[:].rearrange("p (c o) -> p c o", o=1),
                        in0=G2r[:, :, 64:65],
                        in1=G2r[:, :, 65:66], op=ALU.add)
                    nc.vector.tensor_tensor(
                        out=e2[:], in0=e2[:],
                        in1=fd2e_sb[:, bl * cpe:(bl + 1) * cpe], op=ALU.add)
                    if use_prelu:
                        nc.scalar.activation(out=lr2[:], in_=e2[:],
                                             func=AF.Prelu, alpha=ALPHA)
                    else:
                        nc.vector.tensor_scalar_mul(lr2[:], e2[:], ALPHA)
                        nc.vector.tensor_tensor(out=lr2[:], in0=e2[:],
                                                in1=lr2[:], op=ALU.max)
                    nc.scalar.activation(out=p2[:], in_=lr2[:], func=AF.Exp)
                    R2 = pc.tile([128, cpe * 65], f16d, tag="R2")
                    R2r = R2[:].rearrange("p (c d) -> p c d", d=65)
                    nc.vector.tensor_tensor(
                        out=R2r[:, :, 0:64],
                        in0=G2r[:, :, 0:64],
                        in1=p2[:].to_broadcast([128, cpe, 64]),
                        op=ALU.mult)
                    nc.vector.tensor_copy(
                        out=R2r[:, :, 64:65],
                        in_=p2[:].rearrange("p (c o) -> p c o", o=1))
                    for i in range(cpe):
                        nc.tensor.matmul(out=ps3[:],
                                         lhsT=OH2[:, i * 128:(i + 1) * 128],
                                         rhs=R2[:, i * 65:(i + 1) * 65],
                                         start=(i == 0), stop=(i == cpe - 1))
                    d2c = pc.tile([128, 1], f32d, tag="d2c")
                    nc.vector.tensor_scalar_max(d2c[:], ps3[:, 64:65], 1e-30)
                    nc.vector.reciprocal(out=d2c[:], in_=d2c[:])
                    o = pc.tile([128, 64], f32d, tag="o")
                    nc.vector.tensor_tensor(
                        out=o[:].rearrange("p (c f) -> p c f", c=1),
                        in0=ps3[:, 0:64].rearrange("p (c f) -> p c f", c=1),
                        in1=d2c[:].to_broadcast([128, 1, 64]),
                        op=ALU.mult)
                    nc.sync.dma_start(out=out[rows, :], in_=o[:])

    nc.compile()
    _BUILD_CACHE[key] = nc
    return nc


def kernel(**inputs):
    global LAST_EXEC_NS, LAST_RESULTS
    x = inputs["x"].astype(np.float32)
    row = inputs["row"].astype(np.int64)
    col = inputs["col"].astype(np.int64)
    W, a = inputs["W"].astype(np.float32), inputs["a"].astype(np.float32)
    W_out = inputs["W_out"].astype(np.float32)
    a_out = inputs["a_out"].astype(np.float32)

    cilo, cihi, oht, ohh, cpl, cph = _preprocess(row, col)

    # head-MINOR (f,h) feature order for layer-1 Wh and layer-2 rows
    W_cat = np.stack([W[h] for h in range(NHEADS)], axis=-1)  # [in, f, h]
    W_cat = W_cat.reshape(NFEAT, NHID * NHEADS)
    WA_dst = np.stack([W[h] @ a[h, :NHID] for h in range(NHEADS)], 1)
    WA_src = np.stack([W[h] @ a[h, NHID:] for h in range(NHEADS)], 1)
    w1_np = np.concatenate([W_cat, WA_dst, WA_src], 1).astype(np.float16)
    w2full = np.concatenate([W_out, (W_out @ a_out[:NCLASS])[:, None],
                             (W_out @ a_out[NCLASS:])[:, None]], 1)
    idx = np.arange(NHID * NHEADS)
    perm = (idx % NHEADS) * NHID + idx // NHEADS   # (f,h) -> h*64+f
    w2_np = w2full[perm, :].astype(np.float16)

    x_pad = np.zeros((NPAD, NFEAT), np.float16)
    x_pad[:N] = x

    nc = _build(cpl, cph)

    fp8 = bool(int(os.environ.get("GAT_FP8_OH", "1")))
    ohdt = mybir.dt.np(f8d) if fp8 else np.float16
    in_maps = []
    for c in range(NC):
        xs = x_pad[c * SHARD:(c + 1) * SHARD]            # [6272, 512]
        xt = (xs.reshape(NBPC, 128, KT, 128)             # [nt, n, k, f]
                .transpose(0, 3, 2, 1)                   # [nt, f, k, n]
                .reshape(NBPC, 128, KT * 128)).copy()
        in_maps.append({"xt": xt, "w1": w1_np, "w2": w2_np,
                        "cilo": cilo[c], "cihi": cihi[c],
                        "ohtT": oht[c].astype(ohdt),
                        "ohhT": ohh[c].astype(ohdt)})

    trace = bool(int(os.environ.get("GAT_TRACE", "0")))
    res = run_bass_kernel_spmd(nc, in_maps, list(range(NC)), trace=trace,
                               trace_cores=list(range(NC)) if trace else None)
    LAST_EXEC_NS = res.exec_time_ns
    LAST_RESULTS = res
    outs = [res.results[c]["out"] for c in range(NC)]
    return np.concatenate(outs, 0)[:N].astype(np.float32)


# revision 26
# speedup vs baseline: 15043.6473x; 1.0600x over previous
"""GAT (2-layer, 8-head) Trainium2 kernel over 8 NeuronCores.

Strategy (edge-cut node sharding), v2:
- Pad N 50000->50176 = 8 shards * 6272. Core c owns nodes [6272c, 6272(c+1)).
- Host: sort edges by dest, bucket into 128-node blocks. Within a block, edges
  are split lo (shard-row < HSPL) / hi, because the batched DMA_GATHER takes
  int16 indices (<=32767) and the full table has 50176 rows. HSPL=2688 is
  128-aligned so the A/B table sections align to 128-row tiles.
- Table-1 row: [Wh in (f,h) head-MINOR order 512 | f_src f16 8 | f_src
  residual 8 | pad -> 640] (1280B, elem %256B). Head-minor order makes the
  per-edge attention scale R = p (*) Wh a DVE TENSOR_TENSOR with an OUTER-dim
  broadcast and innermost stride 1 -> 2x_1P perf mode (the head-major layout
  forces a stride-0 innermost broadcast = 1x).
- One dma_gather per (block, section) (fixed SWDGE overhead ~1us dominates,
  so fewer+bigger gathers), round-robin over 4 queues.
- One-hots (OH: [edge,dst], OHT: [dst,edge]) shipped as fp8e4 (0/1 exact):
  halves their HBM traffic and speeds LDWEIGHTS via FWL.
- leaky-relu on the Scalar engine via Prelu(alpha) (same act table set as Exp
  and Copy -> no table reloads); ELU via hm=Exp(ha) then min/add tensor_scalar
  + max (exp is monotone: e^min(x,0) == min(e^x, 1)).
- AllGathers chunked into 3 pieces per table section and interleaved into the
  producing phase so the collective overlaps compute.
- Softmax needs no segment-max: logits are O(6) so exp never overflows, and
  normalization commutes with the scatter-sum (divide once per node).
"""
import os
import sys
sys.path.insert(0, "/opt/trn_rl_repo")
import numpy as np

import concourse.tile as tile
from concourse import bass, bacc, mybir
from concourse.bass_utils import run_bass_kernel_spmd
from concourse.masks import make_identity

N, E = 50000, 800000
NFEAT, NHID, NHEADS, NCLASS = 512, 64, 8, 64
ALPHA = 0.2
NC = 8
NPAD = 50176
SHARD = NPAD // NC        # 6272
BLK = 128
NBPC = SHARD // BLK       # 49 blocks per core
KT = NFEAT // 128         # 4 k-tiles
DW1 = NFEAT + 16          # 528: Wh | src | src_residual
TW1 = 640                 # table-1 row (1280B, %256B for dma_gather)
DW2 = NCLASS + 2          # 66:  Wh2 | src | src_residual
TW2 = 128                 # table-2 row (256B)
HSPL = 2688               # 21*128; section A rows per shard (8*2688<=32767)
TSPL = HSPL // 128        # 21 tiles in section A

f16d, f32d, i16d = mybir.dt.float16, mybir.dt.float32, mybir.dt.int16
f8d = mybir.dt.float8e4

LAST_EXEC_NS = None
LAST_RESULTS = None
_BUILD_CACHE = {}

def _wrap16(lst):
    """DMA_GATHER index layout: element i at [i%16, i//16], replicated x8."""
    return np.tile(lst.reshape(-1, 16).T, (8, 1))


def _preprocess(row, col):
    order = np.argsort(row, kind="stable")
    row_s = row[order].astype(np.int64)
    col_s = col[order].astype(np.int64)
    counts = np.bincount(row_s // BLK, minlength=NPAD // BLK)
    starts = np.concatenate([[0], np.cumsum(counts)])
    nb = NPAD // BLK
    src_c = col_s // SHARD
    src_r = col_s % SHARD
    in_a = src_r < HSPL
    idx_a = src_c * HSPL + src_r
    idx_b = src_c * (SHARD - HSPL) + (src_r - HSPL)
    nlo = np.zeros(nb, np.int64)
    for b in range(nb):
        nlo[b] = int(in_a[starts[b]:starts[b + 1]].sum())
    nhi = counts - nlo
    cpl = int(((nlo + 127) // 128).max())
    cph = int(((nhi + 127) // 128).max())
    cpe = cpl + cph
    cilo = np.zeros((NC, 128, NBPC * cpl * 8), np.int16)
    cihi = np.zeros((NC, 128, NBPC * cph * 8), np.int16)
    oht = np.zeros((NC, NBPC, 128, cpe * 128), np.float16)
    ohh = np.zeros((NC, NBPC, 128, cpe * 128), np.float16)
    iota = np.arange(128)
    for b in range(nb):
        c, bl = divmod(b, NBPC)
        s, e = starts[b], starts[b + 1]
        rloc = row_s[s:e] - b * BLK
        m = in_a[s:e]
        lo_c, lo_r = idx_a[s:e][m], rloc[m]
        hi_c, hi_r = idx_b[s:e][~m], rloc[~m]
        lst = np.zeros(cpl * 128, np.int16)
        lst[:len(lo_c)] = lo_c
        cilo[c, :, bl * cpl * 8:(bl + 1) * cpl * 8] = _wrap16(lst)
        rlo = np.full(cpl * 128, 200.0, np.float16)
        rlo[:len(lo_r)] = lo_r
        lst2 = np.zeros(cph * 128, np.int16)
        lst2[:len(hi_c)] = hi_c
        cihi[c, :, bl * cph * 8:(bl + 1) * cph * 8] = _wrap16(lst2)
        rhi = np.full(cph * 128, 200.0, np.float16)
        rhi[:len(hi_r)] = hi_r
        rf = np.concatenate([rlo, rhi])
        oht[c, bl] = (iota[:, None] == rf[None, :]).astype(np.float16)
        rfs = rf.reshape(cpe, 128)
        for cc in range(cpe):
            ohh[c, bl][:, cc * 128:(cc + 1) * 128] = (
                rfs[cc][:, None] == iota[None, :]).astype(np.float16)
    return cilo, cihi, oht, ohh, cpl, cph


def _build(cpl, cph):
    key = (cpl, cph, os.environ.get("GAT_FP8_OH", "1"),
           os.environ.get("GAT_PRELU", "1"), os.environ.get("GAT_MAXCH", "4"),
           os.environ.get("GAT_MAXCH2", "4"))
    if key in _BUILD_CACHE:
        return _BUILD_CACHE[key]
    cpe = cpl + cph
    ohd = f8d if int(os.environ.get("GAT_FP8_OH", "1")) else f16d
    use_prelu = bool(int(os.environ.get("GAT_PRELU", "1")))
    nc = bacc.Bacc("TRN2", target_bir_lowering=False, debug=False,
                   enable_asserts=True, num_devices=NC, num_swdge_queues=4)
    xt = nc.dram_tensor("xt", [NBPC, 128, KT * 128], f16d, kind="ExternalInput")
    w1 = nc.dram_tensor("w1", [KT * 128, DW1], f16d, kind="ExternalInput")
    w2 = nc.dram_tensor("w2", [KT * 128, DW2], f16d, kind="ExternalInput")
    cilo = nc.dram_tensor("cilo", [128, NBPC * cpl * 8], i16d, kind="ExternalInput")
    cihi = nc.dram_tensor("cihi", [128, NBPC * cph * 8], i16d, kind="ExternalInput")
    ohtT = nc.dram_tensor("ohtT", [NBPC, 128, cpe * 128], ohd,
                          kind="ExternalInput")
    ohhT = nc.dram_tensor("ohhT", [NBPC, 128, cpe * 128], ohd,
                          kind="ExternalInput")
    out = nc.dram_tensor("out", [SHARD, NCLASS], f32d, kind="ExternalOutput")

    AF, ALU = mybir.ActivationFunctionType, mybir.AluOpType
    HSPB = SHARD - HSPL

    qctr = [0]
    # SWDGE dma_gather hangs above 512 idxs/instruction (at 1280B elems)
    maxch = int(os.environ.get("GAT_MAXCH", "4"))
    maxch2 = int(os.environ.get("GAT_MAXCH2", "4"))

    def _gather(dst, table, idx_t, icol0, nch, elem, mx):
        a = 0
        while a < nch:
            b = min(a + mx, nch)
            nc.gpsimd.dma_gather(
                out_ap=dst[:, a * elem:b * elem]
                    .rearrange("p (c e) -> p c e", e=elem),
                in_ap=table,
                idxs_ap=idx_t[:, icol0 + a * 8:icol0 + b * 8],
                num_idxs=(b - a) * 128, num_idxs_reg=(b - a) * 128,
                elem_size=elem, queue_num=qctr[0] % 4)
            qctr[0] += 1
            a = b

    def gather_sect(dst, table, idx_t, icol0, nch, elem):
        _gather(dst, table, idx_t, icol0, nch, elem, maxch)

    def gather_sect2(dst, table, idx_t, icol0, nch, elem):
        _gather(dst, table, idx_t, icol0, nch, elem, maxch2)

    with tile.TileContext(nc) as tc:
        with tc.tile_pool(name="res", bufs=1) as res, \
             tc.tile_pool(name="dram", bufs=1, space="DRAM") as drp:
            # shard tables; section-A collective issues mid-phase (a Shared
            # tensor may only be written by ONE instruction, so chunking
            # finer than per-section is not possible)
            tab1sA = drp.tile([HSPL, TW1], f16d)
            tab1sB = drp.tile([HSPB, TW1], f16d)
            tab2sA = drp.tile([HSPL, TW2], f16d)
            tab2sB = drp.tile([HSPB, TW2], f16d)
            tab1a = drp.tile([NC * HSPL, TW1], f16d, addr_space="Shared")
            tab1b = drp.tile([NC * HSPB, TW1], f16d, addr_space="Shared")
            tab2a = drp.tile([NC * HSPL, TW2], f16d, addr_space="Shared")
            tab2b = drp.tile([NC * HSPB, TW2], f16d, addr_space="Shared")

            def ag_sect(src, dst):
                nc.gpsimd.collective_compute(
                    "AllGather", ALU.bypass, replica_groups=[list(range(NC))],
                    ins=[src.opt()],
                    outs=[dst[:].rearrange("(c r) d -> c r d", c=NC)])

            w1_t = res.tile([128, KT * DW1], f16d)
            w2_t = res.tile([128, KT * DW2], f16d)
            for k in range(KT):
                nc.sync.dma_start(out=w1_t[:, k * DW1:(k + 1) * DW1],
                                  in_=w1[k * 128:(k + 1) * 128, :])
                nc.sync.dma_start(out=w2_t[:, k * DW2:(k + 1) * DW2],
                                  in_=w2[k * 128:(k + 1) * 128, :])
            cilo_t = res.tile([128, NBPC * cpl * 8], i16d)
            cihi_t = res.tile([128, NBPC * cph * 8], i16d)
            nc.sync.dma_start(out=cilo_t[:], in_=cilo[:, :])
            nc.sync.dma_start(out=cihi_t[:], in_=cihi[:, :])
            ident = res.tile([128, 128], f16d)
            make_identity(nc, ident[:])
            fd_sb = res.tile([128, NBPC * 8], f16d)
            fd2_sb = res.tile([128, NBPC], f16d)

            # ---------------- Phase A ----------------
            with nc.named_scope("phaseA"), \
                 tc.tile_pool(name="pa", bufs=5) as pa, \
                 tc.tile_pool(name="ppa", bufs=2, space="PSUM") as ppa:
                def issue_a(nt):
                    xk4 = pa.tile([128, KT * 128], f16d, tag="xk4")
                    nc.sync.dma_start(out=xk4[:], in_=xt[nt, :, :])
                    return xk4
                PFA = 3
                penda = [issue_a(t) for t in range(PFA)]
                for nt in range(NBPC):
                    psA = ppa.tile([128, 512], f32d, tag="psA")
                    psB = ppa.tile([128, 16], f32d, tag="psB")
                    xk4 = penda.pop(0)
                    if nt + PFA < NBPC:
                        penda.append(issue_a(nt + PFA))
                    for k in range(KT):
                        nc.tensor.matmul(out=psA[:],
                                         lhsT=xk4[:, k * 128:(k + 1) * 128],
                                         rhs=w1_t[:, k * DW1:k * DW1 + 512],
                                         start=(k == 0), stop=(k == KT - 1))
                        nc.tensor.matmul(out=psB[:],
                                         lhsT=xk4[:, k * 128:(k + 1) * 128],
                                         rhs=w1_t[:, k * DW1 + 512:(k + 1) * DW1],
                                         start=(k == 0), stop=(k == KT - 1))
                    whf = pa.tile([128, DW1], f16d, tag="whf")
                    nc.vector.tensor_copy(out=whf[:, :512], in_=psA[:])
                    nc.scalar.activation(out=whf[:, 512:520], in_=psB[:, 8:16],
                                         func=AF.Copy)
                    nc.vector.tensor_tensor(out=whf[:, 520:528], in0=psB[:, 8:16],
                                            in1=whf[:, 512:520], op=ALU.subtract)
                    nc.vector.tensor_copy(out=fd_sb[:, nt * 8:(nt + 1) * 8],
                                          in_=psB[:, 0:8])
                    if nt < TSPL:
                        r0 = nt * 128
                        nc.scalar.dma_start(out=tab1sA[r0:r0 + 128, 0:DW1],
                                            in_=whf[:])
                    else:
                        r0 = (nt - TSPL) * 128
                        nc.scalar.dma_start(out=tab1sB[r0:r0 + 128, 0:DW1],
                                            in_=whf[:])
                    if nt == TSPL - 1:
                        with nc.named_scope("ag1"):
                            ag_sect(tab1sA, tab1a)
                    if nt == NBPC - 1:
                        with nc.named_scope("ag1"):
                            ag_sect(tab1sB, tab1b)

            # ---------------- Phase B ----------------
            # 2-stage software pipeline: stage1(bl) = logits+R (DVE/ACT +
            # small PE), stage2(bl) = scatter/elu/layer-2 (big PE + DVE/ACT).
            # Interleaving stage1(i) with stage2(i-1) keeps every in-order
            # engine queue supplied with ready work (no head-of-line stalls).
            with nc.named_scope("phaseB"), \
                 tc.tile_pool(name="pb", bufs=2) as pb, \
                 tc.tile_pool(name="pr", bufs=3) as pr, \
                 tc.tile_pool(name="pgg", bufs=4) as pgg, \
                 tc.tile_pool(name="pgo", bufs=5) as pgo, \
                 tc.tile_pool(name="ppb", bufs=2, space="PSUM") as ppb, \
                 tc.tile_pool(name="ppf", bufs=3, space="PSUM") as ppf, \
                 tc.tile_pool(name="ppt", bufs=2, space="PSUM") as ppt:
                def issue_b(bl):
                    G = pgg.tile([128, cpe * TW1], f16d, tag="G")
                    OH = pgo.tile([128, cpe * 128], ohd, tag="OH")
                    OHT = pgo.tile([128, cpe * 128], ohd, tag="OHT")
                    nc.sync.dma_start(out=OHT[:], in_=ohtT[bl, :, :])
                    nc.sync.dma_start(out=OH[:], in_=ohhT[bl, :, :])
                    gather_sect(G[:, :cpl * TW1], tab1a[:, :],
                                cilo_t, bl * cpl * 8, cpl, TW1)
                    gather_sect(G[:, cpl * TW1:], tab1b[:, :],
                                cihi_t, bl * cph * 8, cph, TW1)
                    return G, OH, OHT

                PF = 3
                pend = [issue_b(b) for b in range(PF)]

                def stage1(bl):
                    G, OH, OHT = pend.pop(0)
                    if bl + PF < NBPC:
                        pend.append(issue_b(bl + PF))
                    aux = ppf.tile([128, 8 + cpe * 8 + DW2], f32d, tag="aux")
                    pfd = aux[:, 8:8 + cpe * 8]
                    for c in range(cpe):
                        nc.tensor.matmul(out=pfd[:, c * 8:(c + 1) * 8],
                                         lhsT=OHT[:, c * 128:(c + 1) * 128],
                                         rhs=fd_sb[:, bl * 8:(bl + 1) * 8],
                                         start=True, stop=True)
                    e1 = pb.tile([128, cpe * 8], f32d, tag="e1")
                    p16 = pr.tile([128, cpe * 8], f16d, tag="p16")
                    Gr = G[:].rearrange("p (c d) -> p c d", d=TW1)
                    nc.vector.tensor_tensor(
                        out=e1[:].rearrange("p (c f) -> p c f", c=cpe),
                        in0=Gr[:, :, 512:520],
                        in1=Gr[:, :, 520:528], op=ALU.add)
                    nc.vector.tensor_tensor(out=e1[:], in0=e1[:], in1=pfd,
                                            op=ALU.add)
                    if use_prelu:
                        nc.scalar.activation(out=e1[:], in_=e1[:],
                                             func=AF.Prelu, alpha=ALPHA)
                    else:
                        lr = pb.tile([128, cpe * 8], f32d, tag="lr")
                        nc.vector.tensor_scalar_mul(lr[:], e1[:], ALPHA)
                        nc.vector.tensor_tensor(out=e1[:], in0=e1[:],
                                                in1=lr[:], op=ALU.max)
                    nc.scalar.activation(out=p16[:], in_=e1[:], func=AF.Exp)
                    # R[p, c, f, h] = Wh[p, c, f, h] * p16[p, c, h] -- the
                    # h-minor layout keeps innermost stride 1 => DVE 2x mode
                    R = pr.tile([128, cpe * 512], f16d, tag="R")
                    nc.vector.tensor_tensor(
                        out=R[:].rearrange("p (c f h) -> p c f h", f=64, h=8),
                        in0=Gr[:, :, 0:512]
                            .rearrange("p c (f h) -> p c f h", f=64),
                        in1=p16[:].rearrange("p (c o h) -> p c o h", o=1, h=8)
                            .to_broadcast([128, cpe, 64, 8]),
                        op=ALU.mult)
                    return (bl, OH, aux, R, p16)

                def stage2(st):
                    bl, OH, aux, R, p16 = st
                    pden = aux[:, 0:8]
                    ps2 = aux[:, 8 + cpe * 8:8 + cpe * 8 + DW2]
                    pnum = ppb.tile([128, 512], f32d, tag="pnum")
                    for i in range(cpe):
                        nc.tensor.matmul(out=pnum[:],
                                         lhsT=OH[:, i * 128:(i + 1) * 128],
                                         rhs=R[:, i * 512:(i + 1) * 512],
                                         start=(i == 0), stop=(i == cpe - 1))
                        nc.tensor.matmul(out=pden,
                                         lhsT=OH[:, i * 128:(i + 1) * 128],
                                         rhs=p16[:, i * 8:(i + 1) * 8],
                                         start=(i == 0), stop=(i == cpe - 1))
                    dcl = pb.tile([128, 8], f32d, tag="dcl")
                    nc.vector.tensor_scalar_max(dcl[:], pden, 1e-30)
                    nc.vector.reciprocal(out=dcl[:], in_=dcl[:])
                    ha = pb.tile([128, 512], f16d, tag="ha")
                    nc.vector.tensor_tensor(
                        out=ha[:].rearrange("p (f h) -> p f h", f=64),
                        in0=pnum[:].rearrange("p (f h) -> p f h", f=64),
                        in1=dcl[:].rearrange("p (o h) -> p o h", o=1)
                            .to_broadcast([128, 64, 8]),
                        op=ALU.mult)
                    # elu: h16 = max(ha, min(e^ha, 1) - 1)  (exp monotone)
                    hm = pb.tile([128, 512], f16d, tag="hm")
                    nc.scalar.activation(out=hm[:], in_=ha[:], func=AF.Exp)
                    nc.vector.tensor_scalar(out=hm[:], in0=hm[:],
                                            scalar1=1.0, scalar2=-1.0,
                                            op0=ALU.min, op1=ALU.add)
                    h16 = pb.tile([128, 512], f16d, tag="h16")
                    nc.vector.tensor_tensor(out=h16[:], in0=hm[:], in1=ha[:],
                                            op=ALU.max)
                    for k in range(KT):
                        pt = ppt.tile([128, 128], f16d, tag="pt")
                        nc.tensor.transpose(out=pt[:],
                                            in_=h16[:, k * 128:(k + 1) * 128],
                                            identity=ident[:])
                        ht = pb.tile([128, 128], f16d, tag="ht")
                        if k % 2 == 0:
                            nc.vector.tensor_copy(out=ht[:], in_=pt[:])
                        else:
                            nc.scalar.activation(out=ht[:], in_=pt[:],
                                                 func=AF.Copy)
                        nc.tensor.matmul(out=ps2, lhsT=ht[:],
                                         rhs=w2_t[:, k * DW2:(k + 1) * DW2],
                                         start=(k == 0), stop=(k == KT - 1))
                    t2 = pb.tile([128, DW2], f16d, tag="t2")
                    nc.scalar.activation(out=t2[:, 0:64], in_=ps2[:, 0:64],
                                         func=AF.Copy)
                    nc.scalar.activation(out=t2[:, 64:65], in_=ps2[:, 65:66],
                                         func=AF.Copy)
                    nc.vector.tensor_tensor(out=t2[:, 65:66], in0=ps2[:, 65:66],
                                            in1=t2[:, 64:65], op=ALU.subtract)
                    nc.scalar.activation(out=fd2_sb[:, bl:bl + 1],
                                         in_=ps2[:, 64:65], func=AF.Copy)
                    if bl < TSPL:
                        r0 = bl * 128
                        nc.scalar.dma_start(out=tab2sA[r0:r0 + 128, 0:DW2],
                                            in_=t2[:])
                    else:
                        r0 = (bl - TSPL) * 128
                        nc.scalar.dma_start(out=tab2sB[r0:r0 + 128, 0:DW2],
                                            in_=t2[:])
                    if bl == TSPL - 1:
                        with nc.named_scope("ag2"):
                            ag_sect(tab2sA, tab2a)
                    if bl == NBPC - 1:
                        with nc.named_scope("ag2"):
                            ag_sect(tab2sB, tab2b)

                live = []
                for bl in range(NBPC):
                    live.append(stage1(bl))
                    if len(live) > 1:
                        stage2(live.pop(0))
                stage2(live.pop(0))

            # ---------------- Phase C ----------------
            with nc.named_scope("phaseC"), \
                 tc.tile_pool(name="pc", bufs=2) as pc, \
                 tc.tile_pool(name="pr2", bufs=3) as pr2, \
                 tc.tile_pool(name="pg2", bufs=4) as pg2, \
                 tc.tile_pool(name="pgo2", bufs=5) as pgo2, \
                 tc.tile_pool(name="ppc", bufs=3, space="PSUM") as ppc:
                def issue_c(bl):
                    G2 = pg2.tile([128, cpe * TW2], f16d, tag="G2")
                    OH2 = pgo2.tile([128, cpe * 128], ohd, tag="OH2")
                    OH2T = pgo2.tile([128, cpe * 128], ohd, tag="OH2T")
                    nc.sync.dma_start(out=OH2[:], in_=ohhT[bl, :, :])
                    nc.sync.dma_start(out=OH2T[:], in_=ohtT[bl, :, :])
                    gather_sect2(G2[:, :cpl * TW2], tab2a[:, :],
                                 cilo_t, bl * cpl * 8, cpl, TW2)
                    gather_sect2(G2[:, cpl * TW2:], tab2b[:, :],
                                 cihi_t, bl * cph * 8, cph, TW2)
                    return G2, OH2, OH2T

                PF = 3
                pend2 = [issue_c(b) for b in range(PF)]

                def stage1c(bl):
                    G2, OH2, OH2T = pend2.pop(0)
                    if bl + PF < NBPC:
                        pend2.append(issue_c(bl + PF))
                    pfd2 = ppc.tile([128, cpe], f32d, tag="pfd2")
                    for c in range(cpe):
                        nc.tensor.matmul(out=pfd2[:, c:c + 1],
                                         lhsT=OH2T[:, c * 128:(c + 1) * 128],
                                         rhs=fd2_sb[:, bl:bl + 1],
                                         start=True, stop=True)
                    e2 = pc.tile([128, cpe], f32d, tag="e2")
                    p2 = pr2.tile([128, cpe], f16d, tag="p2")
                    G2r = G2[:].rearrange("p (c d) -> p c d", d=TW2)
                    nc.vector.tensor_tensor(
                        out=e2[:].rearrange("p (c o) -> p c o", o=1),
                        in0=G2r[:, :, 64:65],
                        in1=G2r[:, :, 65:66], op=ALU.add)
                    nc.vector.tensor_tensor(
                        out=e2[:], in0=e2[:], in1=pfd2[:], op=ALU.add)
                    if use_prelu:
                        nc.scalar.activation(out=e2[:], in_=e2[:],
                                             func=AF.Prelu, alpha=ALPHA)
                    else:
                        lr2 = pc.tile([128, cpe], f32d, tag="lr2")
                        nc.vector.tensor_scalar_mul(lr2[:], e2[:], ALPHA)
                        nc.vector.tensor_tensor(out=e2[:], in0=e2[:],
                                                in1=lr2[:], op=ALU.max)
                    nc.scalar.activation(out=p2[:], in_=e2[:], func=AF.Exp)
                    R2 = pr2.tile([128, cpe * 65], f16d, tag="R2")
                    R2r = R2[:].rearrange("p (c d) -> p c d", d=65)
                    nc.vector.tensor_tensor(
                        out=R2r[:, :, 0:64],
                        in0=G2r[:, :, 0:64],
                        in1=p2[:].to_broadcast([128, cpe, 64]),
                        op=ALU.mult)
                    nc.vector.tensor_copy(
                        out=R2r[:, :, 64:65],
                        in_=p2[:].rearrange("p (c o) -> p c o", o=1))
                    return (bl, OH2, R2)

                def stage2c(st):
                    bl, OH2, R2 = st
                    rows = slice(bl * 128, (bl + 1) * 128)
                    ps3 = ppc.tile([128, 65], f32d, tag="ps3")
                    for i in range(cpe):
                        nc.tensor.matmul(out=ps3[:],
                                         lhsT=OH2[:, i * 128:(i + 1) * 128],
                                         rhs=R2[:, i * 65:(i + 1) * 65],
                                         start=(i == 0), stop=(i == cpe - 1))
                    d2c = pc.tile([128, 1], f32d, tag="d2c")
                    nc.vector.tensor_scalar_max(d2c[:], ps3[:, 64:65], 1e-30)
                    nc.vector.reciprocal(out=d2c[:], in_=d2c[:])
                    o = pc.tile([128, 64], f32d, tag="o")
                    nc.vector.tensor_tensor(
                        out=o[:].rearrange("p (c f) -> p c f", c=1),
                        in0=ps3[:, 0:64].rearrange("p (c f) -> p c f", c=1),
                        in1=d2c[:].to_broadcast([128, 1, 64]),
                        op=ALU.mult)
                    nc.scalar.dma_start(out=out[rows, :], in_=o[:])

                live2 = []
                for bl in range(NBPC):
                    live2.append(stage1c(bl))
                    if len(live2) > 1:
                        stage2c(live2.pop(0))
                stage2c(live2.pop(0))

    nc.compile()
    _BUILD_CACHE[key] = nc
    return nc


def kernel(**inputs):
    global LAST_EXEC_NS, LAST_RESULTS
    x = inputs["x"].astype(np.float32)
    row = inputs["row"].astype(np.int64)
    col = inputs["col"].astype(np.int64)
    W, a = inputs["W"].astype(np.float32), inputs["a"].astype(np.float32)
    W_out = inputs["W_out"].astype(np.float32)
    a_out = inputs["a_out"].astype(np.float32)

    cilo, cihi, oht, ohh, cpl, cph = _preprocess(row, col)

    # head-MINOR (f,h) feature order for layer-1 Wh and layer-2 rows
    W_cat = np.stack([W[h] for h in range(NHEADS)], axis=-1)  # [in, f, h]
    W_cat = W_cat.reshape(NFEAT, NHID * NHEADS)
    WA_dst = np.stack([W[h] @ a[h, :NHID] for h in range(NHEADS)], 1)
    WA_src = np.stack([W[h] @ a[h, NHID:] for h in range(NHEADS)], 1)
    w1_np = np.concatenate([W_cat, WA_dst, WA_src], 1).astype(np.float16)
    w2full = np.concatenate([W_out, (W_out @ a_out[:NCLASS])[:, None],
                             (W_out @ a_out[NCLASS:])[:, None]], 1)
    idx = np.arange(NHID * NHEADS)
    perm = (idx % NHEADS) * NHID + idx // NHEADS   # (f,h) -> h*64+f
    w2_np = w2full[perm, :].astype(np.float16)

    x_pad = np.zeros((NPAD, NFEAT), np.float16)
    x_pad[:N] = x

    nc = _build(cpl, cph)

    fp8 = bool(int(os.environ.get("GAT_FP8_OH", "1")))
    ohdt = mybir.dt.np(f8d) if fp8 else np.float16
    in_maps = []
    for c in range(NC):
        xs = x_pad[c * SHARD:(c + 1) * SHARD]            # [6272, 512]
        xt = (xs.reshape(NBPC, 128, KT, 128)             # [nt, n, k, f]
                .transpose(0, 3, 2, 1)                   # [nt, f, k, n]
                .reshape(NBPC, 128, KT * 128)).copy()
        in_maps.append({"xt": xt, "w1": w1_np, "w2": w2_np,
                        "cilo": cilo[c], "cihi": cihi[c],
                        "ohtT": oht[c].astype(ohdt),
                        "ohhT": ohh[c].astype(ohdt)})

    trace = bool(int(os.environ.get("GAT_TRACE", "0")))
    res = run_bass_kernel_spmd(nc, in_maps, list(range(NC)), trace=trace,
                               trace_cores=list(range(NC)) if trace else None)
    LAST_EXEC_NS = res.exec_time_ns
    LAST_RESULTS = res
    outs = [res.results[c]["out"] for c in range(NC)]
    return np.concatenate(outs, 0)[:N].astype(np.float32)


# revision 28
# speedup vs baseline: 16829.5586x; 1.1187x over previous
"""GAT (2-layer, 8-head) Trainium2 kernel over 8 NeuronCores.

Strategy (edge-cut node sharding), v2:
- Pad N 50000->50176 = 8 shards * 6272. Core c owns nodes [6272c, 6272(c+1)).
- Host: sort edges by dest, bucket into 128-node blocks. Within a block, edges
  are split lo (shard-row < HSPL) / hi, because the batched DMA_GATHER takes
  int16 indices (<=32767) and the full table has 50176 rows. HSPL=2688 is
  128-aligned so the A/B table sections align to 128-row tiles.
- Table-1 row: [Wh in (f,h) head-MINOR order 512 | f_src f16 8 | f_src
  residual 8 | pad -> 640] (1280B, elem %256B). Head-minor order makes the
  per-edge attention scale R = p (*) Wh a DVE TENSOR_TENSOR with an OUTER-dim
  broadcast and innermost stride 1 -> 2x_1P perf mode (the head-major layout
  forces a stride-0 innermost broadcast = 1x).
- One dma_gather per (block, section) (fixed SWDGE overhead ~1us dominates,
  so fewer+bigger gathers), round-robin over 4 queues.
- One-hots (OH: [edge,dst], OHT: [dst,edge]) shipped as fp8e4 (0/1 exact):
  halves their HBM traffic and speeds LDWEIGHTS via FWL.
- leaky-relu on the Scalar engine via Prelu(alpha) (same act table set as Exp
  and Copy -> no table reloads); ELU via hm=Exp(ha) then min/add tensor_scalar
  + max (exp is monotone: e^min(x,0) == min(e^x, 1)).
- AllGathers chunked into 3 pieces per table section and interleaved into the
  producing phase so the collective overlaps compute.
- Softmax needs no segment-max: logits are O(6) so exp never overflows, and
  normalization commutes with the scatter-sum (divide once per node).
"""
import os
import sys
sys.path.insert(0, "/opt/trn_rl_repo")
import numpy as np

import concourse.tile as tile
from concourse import bass, bacc, mybir
from concourse.bass_utils import run_bass_kernel_spmd
from concourse.masks import make_identity

N, E = 50000, 800000
NFEAT, NHID, NHEADS, NCLASS = 512, 64, 8, 64
ALPHA = 0.2
NC = 8
NPAD = 50176
SHARD = NPAD // NC        # 6272
BLK = 128
NBPC = SHARD // BLK       # 49 blocks per core
KT = NFEAT // 128         # 4 k-tiles
DW1 = NFEAT + 16          # 528: Wh | src | src_residual (w1 matmul width)
TW1 = 768                 # table-1 row BYTES: 512 f8e3 Wh | 8+8 f16 fs | pad
DW2 = NCLASS + 2          # 66:  Wh2 | src | src_residual
TW2 = 128                 # table-2 row (256B)
HSPL = 2688               # 21*128; section A rows per shard (8*2688<=32767)
TSPL = HSPL // 128        # 21 tiles in section A

f16d, f32d, i16d = mybir.dt.float16, mybir.dt.float32, mybir.dt.int16
f8d = mybir.dt.float8e4
f8e3 = mybir.dt.float8e3

LAST_EXEC_NS = None
LAST_RESULTS = None
_BUILD_CACHE = {}

def _wrap16(lst):
    """DMA_GATHER index layout: element i at [i%16, i//16], replicated x8."""
    return np.tile(lst.reshape(-1, 16).T, (8, 1))


def _preprocess(row, col):
    order = np.argsort(row, kind="stable")
    row_s = row[order].astype(np.int64)
    col_s = col[order].astype(np.int64)
    counts = np.bincount(row_s // BLK, minlength=NPAD // BLK)
    starts = np.concatenate([[0], np.cumsum(counts)])
    nb = NPAD // BLK
    src_c = col_s // SHARD
    src_r = col_s % SHARD
    in_a = src_r < HSPL
    idx_a = src_c * HSPL + src_r
    idx_b = src_c * (SHARD - HSPL) + (src_r - HSPL)
    nlo = np.zeros(nb, np.int64)
    for b in range(nb):
        nlo[b] = int(in_a[starts[b]:starts[b + 1]].sum())
    nhi = counts - nlo
    cpl = int(((nlo + 127) // 128).max())
    cph = int(((nhi + 127) // 128).max())
    cpe = cpl + cph
    cilo = np.zeros((NC, 128, NBPC * cpl * 8), np.int16)
    cihi = np.zeros((NC, 128, NBPC * cph * 8), np.int16)
    oht = np.zeros((NC, NBPC, 128, cpe * 128), np.float16)
    ohh = np.zeros((NC, NBPC, 128, cpe * 128), np.float16)
    iota = np.arange(128)
    for b in range(nb):
        c, bl = divmod(b, NBPC)
        s, e = starts[b], starts[b + 1]
        rloc = row_s[s:e] - b * BLK
        m = in_a[s:e]
        lo_c, lo_r = idx_a[s:e][m], rloc[m]
        hi_c, hi_r = idx_b[s:e][~m], rloc[~m]
        lst = np.zeros(cpl * 128, np.int16)
        lst[:len(lo_c)] = lo_c
        cilo[c, :, bl * cpl * 8:(bl + 1) * cpl * 8] = _wrap16(lst)
        rlo = np.full(cpl * 128, 200.0, np.float16)
        rlo[:len(lo_r)] = lo_r
        lst2 = np.zeros(cph * 128, np.int16)
        lst2[:len(hi_c)] = hi_c
        cihi[c, :, bl * cph * 8:(bl + 1) * cph * 8] = _wrap16(lst2)
        rhi = np.full(cph * 128, 200.0, np.float16)
        rhi[:len(hi_r)] = hi_r
        rf = np.concatenate([rlo, rhi])
        oht[c, bl] = (iota[:, None] == rf[None, :]).astype(np.float16)
        rfs = rf.reshape(cpe, 128)
        for cc in range(cpe):
            ohh[c, bl][:, cc * 128:(cc + 1) * 128] = (
                rfs[cc][:, None] == iota[None, :]).astype(np.float16)
    return cilo, cihi, oht, ohh, cpl, cph


def _build(cpl, cph):
    key = (cpl, cph, os.environ.get("GAT_FP8_OH", "1"),
           os.environ.get("GAT_PRELU", "1"), os.environ.get("GAT_MAXCH", "4"),
           os.environ.get("GAT_MAXCH2", "4"), os.environ.get("GAT_CSPL", "15"))
    if key in _BUILD_CACHE:
        return _BUILD_CACHE[key]
    cpe = cpl + cph
    ohd = f8d if int(os.environ.get("GAT_FP8_OH", "1")) else f16d
    use_prelu = bool(int(os.environ.get("GAT_PRELU", "1")))
    nc = bacc.Bacc("TRN2", target_bir_lowering=False, debug=False,
                   enable_asserts=True, num_devices=NC, num_swdge_queues=4)
    xt = nc.dram_tensor("xt", [NBPC, 128, KT * 128], f16d, kind="ExternalInput")
    w1 = nc.dram_tensor("w1", [KT * 128, DW1], f16d, kind="ExternalInput")
    w2 = nc.dram_tensor("w2", [KT * 128, DW2], f16d, kind="ExternalInput")
    cilo = nc.dram_tensor("cilo", [128, NBPC * cpl * 8], i16d, kind="ExternalInput")
    cihi = nc.dram_tensor("cihi", [128, NBPC * cph * 8], i16d, kind="ExternalInput")
    ohtT = nc.dram_tensor("ohtT", [NBPC, 128, cpe * 128], ohd,
                          kind="ExternalInput")
    ohhT = nc.dram_tensor("ohhT", [NBPC, 128, cpe * 128], ohd,
                          kind="ExternalInput")
    out = nc.dram_tensor("out", [SHARD, NCLASS], f32d, kind="ExternalOutput")

    AF, ALU = mybir.ActivationFunctionType, mybir.AluOpType
    HSPB = SHARD - HSPL

    qctr = [0]
    # SWDGE dma_gather hangs above 512 idxs/instruction (at 1280B elems)
    maxch = int(os.environ.get("GAT_MAXCH", "4"))
    CSPL = min(cpl + cph, int(os.environ.get("GAT_CSPL", "15")))
    maxch2 = int(os.environ.get("GAT_MAXCH2", "4"))

    def _gather(dst, table, idx_t, icol0, nch, elem, mx):
        a = 0
        while a < nch:
            b = min(a + mx, nch)
            nc.gpsimd.dma_gather(
                out_ap=dst[:, a * elem:b * elem]
                    .rearrange("p (c e) -> p c e", e=elem),
                in_ap=table,
                idxs_ap=idx_t[:, icol0 + a * 8:icol0 + b * 8],
                num_idxs=(b - a) * 128, num_idxs_reg=(b - a) * 128,
                elem_size=elem, queue_num=qctr[0] % 4)
            qctr[0] += 1
            a = b

    def gather_sect(dst, table, idx_t, icol0, nch, elem):
        _gather(dst, table, idx_t, icol0, nch, elem, maxch)

    def gather_sect2(dst, table, idx_t, icol0, nch, elem):
        _gather(dst, table, idx_t, icol0, nch, elem, maxch2)

    with tile.TileContext(nc) as tc:
        with tc.tile_pool(name="res", bufs=1) as res, \
             tc.tile_pool(name="dram", bufs=1, space="DRAM") as drp:
            # shard tables; section-A collective issues mid-phase (a Shared
            # tensor may only be written by ONE instruction, so chunking
            # finer than per-section is not possible)
            tab1sA = drp.tile([HSPL, TW1], f8e3)
            tab1sB = drp.tile([HSPB, TW1], f8e3)
            tab2sA = drp.tile([HSPL, TW2], f16d)
            tab2sB = drp.tile([HSPB, TW2], f16d)
            tab1a = drp.tile([NC * HSPL, TW1], f8e3, addr_space="Shared")
            tab1b = drp.tile([NC * HSPB, TW1], f8e3, addr_space="Shared")
            tab2a = drp.tile([NC * HSPL, TW2], f16d, addr_space="Shared")
            tab2b = drp.tile([NC * HSPB, TW2], f16d, addr_space="Shared")

            def ag_sect(src, dst):
                nc.gpsimd.collective_compute(
                    "AllGather", ALU.bypass, replica_groups=[list(range(NC))],
                    ins=[src.opt()],
                    outs=[dst[:].rearrange("(c r) d -> c r d", c=NC)])

            w1_t = res.tile([128, KT * DW1], f16d)
            w2_t = res.tile([128, KT * DW2], f16d)
            for k in range(KT):
                nc.sync.dma_start(out=w1_t[:, k * DW1:(k + 1) * DW1],
                                  in_=w1[k * 128:(k + 1) * 128, :])
                nc.sync.dma_start(out=w2_t[:, k * DW2:(k + 1) * DW2],
                                  in_=w2[k * 128:(k + 1) * 128, :])
            cilo_t = res.tile([128, NBPC * cpl * 8], i16d)
            cihi_t = res.tile([128, NBPC * cph * 8], i16d)
            nc.sync.dma_start(out=cilo_t[:], in_=cilo[:, :])
            nc.sync.dma_start(out=cihi_t[:], in_=cihi[:, :])
            ident = res.tile([128, 128], f16d)
            make_identity(nc, ident[:])
            fd_sb = res.tile([128, NBPC * 8], f16d)
            fd2_sb = res.tile([128, NBPC], f16d)

            # ---------------- Phase A ----------------
            with nc.named_scope("phaseA"), \
                 tc.tile_pool(name="pa", bufs=5) as pa, \
                 tc.tile_pool(name="ppa", bufs=2, space="PSUM") as ppa:
                def issue_a(nt):
                    xk4 = pa.tile([128, KT * 128], f16d, tag="xk4")
                    nc.sync.dma_start(out=xk4[:], in_=xt[nt, :, :])
                    return xk4
                PFA = 3
                penda = [issue_a(t) for t in range(PFA)]
                for nt in range(NBPC):
                    psA = ppa.tile([128, 512], f32d, tag="psA")
                    psB = ppa.tile([128, 16], f32d, tag="psB")
                    xk4 = penda.pop(0)
                    if nt + PFA < NBPC:
                        penda.append(issue_a(nt + PFA))
                    for k in range(KT):
                        nc.tensor.matmul(out=psA[:],
                                         lhsT=xk4[:, k * 128:(k + 1) * 128],
                                         rhs=w1_t[:, k * DW1:k * DW1 + 512],
                                         start=(k == 0), stop=(k == KT - 1))
                        nc.tensor.matmul(out=psB[:],
                                         lhsT=xk4[:, k * 128:(k + 1) * 128],
                                         rhs=w1_t[:, k * DW1 + 512:(k + 1) * DW1],
                                         start=(k == 0), stop=(k == KT - 1))
                    whf = pa.tile([128, TW1], f8e3, tag="whf")
                    wh16 = whf[:].bitcast(f16d)      # [128, TW1//2] f16 view
                    nc.vector.tensor_copy(out=whf[:, :512], in_=psA[:])
                    nc.scalar.activation(out=wh16[:, 256:264], in_=psB[:, 8:16],
                                         func=AF.Copy)
                    nc.vector.tensor_tensor(out=wh16[:, 264:272],
                                            in0=psB[:, 8:16],
                                            in1=wh16[:, 256:264],
                                            op=ALU.subtract)
                    nc.vector.tensor_copy(out=fd_sb[:, nt * 8:(nt + 1) * 8],
                                          in_=psB[:, 0:8])
                    if nt < TSPL:
                        r0 = nt * 128
                        nc.scalar.dma_start(out=tab1sA[r0:r0 + 128, 0:544],
                                            in_=whf[:, 0:544])
                    else:
                        r0 = (nt - TSPL) * 128
                        nc.scalar.dma_start(out=tab1sB[r0:r0 + 128, 0:544],
                                            in_=whf[:, 0:544])
                    if nt == TSPL - 1:
                        with nc.named_scope("ag1"):
                            ag_sect(tab1sA, tab1a)
                    if nt == NBPC - 1:
                        with nc.named_scope("ag1"):
                            ag_sect(tab1sB, tab1b)

            # ---------------- Phase B ----------------
            # 2-stage software pipeline: stage1(bl) = logits+R (DVE/ACT +
            # small PE), stage2(bl) = scatter/elu/layer-2 (big PE + DVE/ACT).
            # Interleaving stage1(i) with stage2(i-1) keeps every in-order
            # engine queue supplied with ready work (no head-of-line stalls).
            with nc.named_scope("phaseB"), \
                 tc.tile_pool(name="pb", bufs=2) as pb, \
                 tc.tile_pool(name="pr", bufs=3) as pr, \
                 tc.tile_pool(name="pgg", bufs=4) as pgg, \
                 tc.tile_pool(name="pgo", bufs=5) as pgo, \
                 tc.tile_pool(name="ppb", bufs=2, space="PSUM") as ppb, \
                 tc.tile_pool(name="ppf", bufs=3, space="PSUM") as ppf, \
                 tc.tile_pool(name="ppt", bufs=2, space="PSUM") as ppt:
                def issue_b(bl):
                    G = pgg.tile([128, cpe * TW1], f8e3, tag="G")
                    OH = pgo.tile([128, cpe * 128], ohd, tag="OH")
                    OHT = pgo.tile([128, cpe * 128], ohd, tag="OHT")
                    nc.sync.dma_start(out=OHT[:], in_=ohtT[bl, :, :])
                    nc.sync.dma_start(out=OH[:], in_=ohhT[bl, :, :])
                    gather_sect(G[:, :cpl * TW1], tab1a[:, :],
                                cilo_t, bl * cpl * 8, cpl, TW1)
                    gather_sect(G[:, cpl * TW1:], tab1b[:, :],
                                cihi_t, bl * cph * 8, cph, TW1)
                    return G, OH, OHT

                PF = 3
                pend = [issue_b(b) for b in range(PF)]

                def stage1(bl):
                    G, OH, OHT = pend.pop(0)
                    if bl + PF < NBPC:
                        pend.append(issue_b(bl + PF))
                    aux = ppf.tile([128, 8 + cpe * 8 + DW2], f32d, tag="aux")
                    pfd = aux[:, 8:8 + cpe * 8]
                    for c in range(cpe):
                        nc.tensor.matmul(out=pfd[:, c * 8:(c + 1) * 8],
                                         lhsT=OHT[:, c * 128:(c + 1) * 128],
                                         rhs=fd_sb[:, bl * 8:(bl + 1) * 8],
                                         start=True, stop=True)
                    e1 = pb.tile([128, cpe * 8], f32d, tag="e1")
                    p16 = pr.tile([128, cpe * 8], f16d, tag="p16")
                    Gr = G[:].rearrange("p (c d) -> p c d", d=TW1)
                    Gf = G[:].bitcast(f16d).rearrange("p (c d) -> p c d",
                                                      d=TW1 // 2)
                    nc.vector.tensor_tensor(
                        out=e1[:].rearrange("p (c f) -> p c f", c=cpe),
                        in0=Gf[:, :, 256:264],
                        in1=Gf[:, :, 264:272], op=ALU.add)
                    nc.vector.tensor_tensor(out=e1[:], in0=e1[:], in1=pfd,
                                            op=ALU.add)
                    if use_prelu:
                        nc.scalar.activation(out=e1[:], in_=e1[:],
                                             func=AF.Prelu, alpha=ALPHA)
                    else:
                        lr = pb.tile([128, cpe * 8], f32d, tag="lr")
                        nc.vector.tensor_scalar_mul(lr[:], e1[:], ALPHA)
                        nc.vector.tensor_tensor(out=e1[:], in0=e1[:],
                                                in1=lr[:], op=ALU.max)
                    nc.scalar.activation(out=p16[:], in_=e1[:], func=AF.Exp)
                    # R[p, c, f, h] = Wh[p, c, f, h] * p16[p, c, h].
                    # Wh is fp8 (1-byte) which forbids DVE 2x mode, so split:
                    # chunks [0, CSPL) are upcast to f16 on the Scalar engine
                    # then multiplied at 2x on DVE; chunks [CSPL, cpe) are
                    # multiplied directly from fp8 at 1x. CSPL balances the
                    # two engines.
                    R = pr.tile([128, cpe * 512], f16d, tag="R")
                    if CSPL > 0:
                        Gc = pr.tile([128, CSPL * 512], f16d, tag="Gc")
                        nc.scalar.activation(
                            out=Gc[:], in_=Gr[:, 0:CSPL, 0:512], func=AF.Copy)
                        nc.vector.tensor_tensor(
                            out=R[:, :CSPL * 512]
                                .rearrange("p (c f h) -> p c f h", f=64, h=8),
                            in0=Gc[:].rearrange("p (c f h) -> p c f h",
                                                f=64, h=8),
                            in1=p16[:, :CSPL * 8]
                                .rearrange("p (c o h) -> p c o h", o=1, h=8)
                                .to_broadcast([128, CSPL, 64, 8]),
                            op=ALU.mult)
                    if CSPL < cpe:
                        nc.vector.tensor_tensor(
                            out=R[:, CSPL * 512:]
                                .rearrange("p (c f h) -> p c f h", f=64, h=8),
                            in0=Gr[:, CSPL:, 0:512]
                                .rearrange("p c (f h) -> p c f h", f=64),
                            in1=p16[:, CSPL * 8:]
                                .rearrange("p (c o h) -> p c o h", o=1, h=8)
                                .to_broadcast([128, cpe - CSPL, 64, 8]),
                            op=ALU.mult)
                    return (bl, OH, aux, R, p16)

                def stage2(st):
                    bl, OH, aux, R, p16 = st
                    pden = aux[:, 0:8]
                    ps2 = aux[:, 8 + cpe * 8:8 + cpe * 8 + DW2]
                    pnum = ppb.tile([128, 512], f32d, tag="pnum")
                    for i in range(cpe):
                        nc.tensor.matmul(out=pnum[:],
                                         lhsT=OH[:, i * 128:(i + 1) * 128],
                                         rhs=R[:, i * 512:(i + 1) * 512],
                                         start=(i == 0), stop=(i == cpe - 1))
                        nc.tensor.matmul(out=pden,
                                         lhsT=OH[:, i * 128:(i + 1) * 128],
                                         rhs=p16[:, i * 8:(i + 1) * 8],
                                         start=(i == 0), stop=(i == cpe - 1))
                    dcl = pb.tile([128, 8], f32d, tag="dcl")
                    nc.vector.tensor_scalar_max(dcl[:], pden, 1e-30)
                    nc.vector.reciprocal(out=dcl[:], in_=dcl[:])
                    ha = pb.tile([128, 512], f16d, tag="ha")
                    nc.vector.tensor_tensor(
                        out=ha[:].rearrange("p (f h) -> p f h", f=64),
                        in0=pnum[:].rearrange("p (f h) -> p f h", f=64),
                        in1=dcl[:].rearrange("p (o h) -> p o h", o=1)
                            .to_broadcast([128, 64, 8]),
                        op=ALU.mult)
                    # elu: h16 = max(ha, min(e^ha, 1) - 1)  (exp monotone)
                    hm = pb.tile([128, 512], f16d, tag="hm")
                    nc.scalar.activation(out=hm[:], in_=ha[:], func=AF.Exp)
                    nc.vector.tensor_scalar(out=hm[:], in0=hm[:],
                                            scalar1=1.0, scalar2=-1.0,
                                            op0=ALU.min, op1=ALU.add)
                    h16 = pb.tile([128, 512], f16d, tag="h16")
                    nc.vector.tensor_tensor(out=h16[:], in0=hm[:], in1=ha[:],
                                            op=ALU.max)
                    for k in range(KT):
                        pt = ppt.tile([128, 128], f16d, tag="pt")
                        nc.tensor.transpose(out=pt[:],
                                            in_=h16[:, k * 128:(k + 1) * 128],
                                            identity=ident[:])
                        ht = pb.tile([128, 128], f16d, tag="ht")
                        if k % 2 == 0:
                            nc.vector.tensor_copy(out=ht[:], in_=pt[:])
                        else:
                            nc.scalar.activation(out=ht[:], in_=pt[:],
                                                 func=AF.Copy)
                        nc.tensor.matmul(out=ps2, lhsT=ht[:],
                                         rhs=w2_t[:, k * DW2:(k + 1) * DW2],
                                         start=(k == 0), stop=(k == KT - 1))
                    t2 = pb.tile([128, DW2], f16d, tag="t2")
                    nc.scalar.activation(out=t2[:, 0:64], in_=ps2[:, 0:64],
                                         func=AF.Copy)
                    nc.scalar.activation(out=t2[:, 64:65], in_=ps2[:, 65:66],
                                         func=AF.Copy)
                    nc.vector.tensor_tensor(out=t2[:, 65:66], in0=ps2[:, 65:66],
                                            in1=t2[:, 64:65], op=ALU.subtract)
                    nc.scalar.activation(out=fd2_sb[:, bl:bl + 1],
                                         in_=ps2[:, 64:65], func=AF.Copy)
                    if bl < TSPL:
                        r0 = bl * 128
                        nc.scalar.dma_start(out=tab2sA[r0:r0 + 128, 0:DW2],
                                            in_=t2[:])
                    else:
                        r0 = (bl - TSPL) * 128
                        nc.scalar.dma_start(out=tab2sB[r0:r0 + 128, 0:DW2],
                                            in_=t2[:])
                    if bl == TSPL - 1:
                        with nc.named_scope("ag2"):
                            ag_sect(tab2sA, tab2a)
                    if bl == NBPC - 1:
                        with nc.named_scope("ag2"):
                            ag_sect(tab2sB, tab2b)

                live = []
                for bl in range(NBPC):
                    live.append(stage1(bl))
                    if len(live) > 1:
                        stage2(live.pop(0))
                stage2(live.pop(0))

            # ---------------- Phase C ----------------
            with nc.named_scope("phaseC"), \
                 tc.tile_pool(name="pc", bufs=2) as pc, \
                 tc.tile_pool(name="pr2", bufs=3) as pr2, \
                 tc.tile_pool(name="pg2", bufs=4) as pg2, \
                 tc.tile_pool(name="pgo2", bufs=5) as pgo2, \
                 tc.tile_pool(name="ppc", bufs=3, space="PSUM") as ppc:
                def issue_c(bl):
                    G2 = pg2.tile([128, cpe * TW2], f16d, tag="G2")
                    OH2 = pgo2.tile([128, cpe * 128], ohd, tag="OH2")
                    OH2T = pgo2.tile([128, cpe * 128], ohd, tag="OH2T")
                    nc.sync.dma_start(out=OH2[:], in_=ohhT[bl, :, :])
                    nc.sync.dma_start(out=OH2T[:], in_=ohtT[bl, :, :])
                    gather_sect2(G2[:, :cpl * TW2], tab2a[:, :],
                                 cilo_t, bl * cpl * 8, cpl, TW2)
                    gather_sect2(G2[:, cpl * TW2:], tab2b[:, :],
                                 cihi_t, bl * cph * 8, cph, TW2)
                    return G2, OH2, OH2T

                PF = 3
                pend2 = [issue_c(b) for b in range(PF)]

                def stage1c(bl):
                    G2, OH2, OH2T = pend2.pop(0)
                    if bl + PF < NBPC:
                        pend2.append(issue_c(bl + PF))
                    pfd2 = ppc.tile([128, cpe], f32d, tag="pfd2")
                    for c in range(cpe):
                        nc.tensor.matmul(out=pfd2[:, c:c + 1],
                                         lhsT=OH2T[:, c * 128:(c + 1) * 128],
                                         rhs=fd2_sb[:, bl:bl + 1],
                                         start=True, stop=True)
                    e2 = pc.tile([128, cpe], f32d, tag="e2")
                    p2 = pr2.tile([128, cpe], f16d, tag="p2")
                    G2r = G2[:].rearrange("p (c d) -> p c d", d=TW2)
                    nc.vector.tensor_tensor(
                        out=e2[:].rearrange("p (c o) -> p c o", o=1),
                        in0=G2r[:, :, 64:65],
                        in1=G2r[:, :, 65:66], op=ALU.add)
                    nc.vector.tensor_tensor(
                        out=e2[:], in0=e2[:], in1=pfd2[:], op=ALU.add)
                    if use_prelu:
                        nc.scalar.activation(out=e2[:], in_=e2[:],
                                             func=AF.Prelu, alpha=ALPHA)
                    else:
                        lr2 = pc.tile([128, cpe], f32d, tag="lr2")
                        nc.vector.tensor_scalar_mul(lr2[:], e2[:], ALPHA)
                        nc.vector.tensor_tensor(out=e2[:], in0=e2[:],
                                                in1=lr2[:], op=ALU.max)
                    nc.scalar.activation(out=p2[:], in_=e2[:], func=AF.Exp)
                    R2 = pr2.tile([128, cpe * 65], f16d, tag="R2")
                    R2r = R2[:].rearrange("p (c d) -> p c d", d=65)
                    nc.vector.tensor_tensor(
                        out=R2r[:, :, 0:64],
                        in0=G2r[:, :, 0:64],
                        in1=p2[:].to_broadcast([128, cpe, 64]),
                        op=ALU.mult)
                    nc.vector.tensor_copy(
                        out=R2r[:, :, 64:65],
                        in_=p2[:].rearrange("p (c o) -> p c o", o=1))
                    return (bl, OH2, R2)

                def stage2c(st):
                    bl, OH2, R2 = st
                    rows = slice(bl * 128, (bl + 1) * 128)
                    ps3 = ppc.tile([128, 65], f32d, tag="ps3")
                    for i in range(cpe):
                        nc.tensor.matmul(out=ps3[:],
                                         lhsT=OH2[:, i * 128:(i + 1) * 128],
                                         rhs=R2[:, i * 65:(i + 1) * 65],
                                         start=(i == 0), stop=(i == cpe - 1))
                    d2c = pc.tile([128, 1], f32d, tag="d2c")
                    nc.vector.tensor_scalar_max(d2c[:], ps3[:, 64:65], 1e-30)
                    nc.vector.reciprocal(out=d2c[:], in_=d2c[:])
                    o = pc.tile([128, 64], f32d, tag="o")
                    nc.vector.tensor_tensor(
                        out=o[:].rearrange("p (c f) -> p c f", c=1),
                        in0=ps3[:, 0:64].rearrange("p (c f) -> p c f", c=1),
                        in1=d2c[:].to_broadcast([128, 1, 64]),
                        op=ALU.mult)
                    nc.scalar.dma_start(out=out[rows, :], in_=o[:])

                live2 = []
                for bl in range(NBPC):
                    live2.append(stage1c(bl))
                    if len(live2) > 1:
                        stage2c(live2.pop(0))
                stage2c(live2.pop(0))

    nc.compile()
    _BUILD_CACHE[key] = nc
    return nc


def kernel(**inputs):
    global LAST_EXEC_NS, LAST_RESULTS
    x = inputs["x"].astype(np.float32)
    row = inputs["row"].astype(np.int64)
    col = inputs["col"].astype(np.int64)
    W, a = inputs["W"].astype(np.float32), inputs["a"].astype(np.float32)
    W_out = inputs["W_out"].astype(np.float32)
    a_out = inputs["a_out"].astype(np.float32)

    cilo, cihi, oht, ohh, cpl, cph = _preprocess(row, col)

    # head-MINOR (f,h) feature order for layer-1 Wh and layer-2 rows
    W_cat = np.stack([W[h] for h in range(NHEADS)], axis=-1)  # [in, f, h]
    W_cat = W_cat.reshape(NFEAT, NHID * NHEADS)
    WA_dst = np.stack([W[h] @ a[h, :NHID] for h in range(NHEADS)], 1)
    WA_src = np.stack([W[h] @ a[h, NHID:] for h in range(NHEADS)], 1)
    w1_np = np.concatenate([W_cat, WA_dst, WA_src], 1).astype(np.float16)
    w2full = np.concatenate([W_out, (W_out @ a_out[:NCLASS])[:, None],
                             (W_out @ a_out[NCLASS:])[:, None]], 1)
    idx = np.arange(NHID * NHEADS)
    perm = (idx % NHEADS) * NHID + idx // NHEADS   # (f,h) -> h*64+f
    w2_np = w2full[perm, :].astype(np.float16)

    x_pad = np.zeros((NPAD, NFEAT), np.float16)
    x_pad[:N] = x

    nc = _build(cpl, cph)

    fp8 = bool(int(os.environ.get("GAT_FP8_OH", "1")))
    ohdt = mybir.dt.np(f8d) if fp8 else np.float16
    in_maps = []
    for c in range(NC):
        xs = x_pad[c * SHARD:(c + 1) * SHARD]            # [6272, 512]
        xt = (xs.reshape(NBPC, 128, KT, 128)             # [nt, n, k, f]
                .transpose(0, 3, 2, 1)                   # [nt, f, k, n]
                .reshape(NBPC, 128, KT * 128)).copy()
        in_maps.append({"xt": xt, "w1": w1_np, "w2": w2_np,
                        "cilo": cilo[c], "cihi": cihi[c],
                        "ohtT": oht[c].astype(ohdt),
                        "ohhT": ohh[c].astype(ohdt)})

    trace = bool(int(os.environ.get("GAT_TRACE", "0")))
    res = run_bass_kernel_spmd(nc, in_maps, list(range(NC)), trace=trace,
                               trace_cores=list(range(NC)) if trace else None)
    LAST_EXEC_NS = res.exec_time_ns
    LAST_RESULTS = res
    outs = [res.results[c]["out"] for c in range(NC)]
    return np.concatenate(outs, 0)[:N].astype(np.float32)


# revision 32
# speedup vs baseline: 17691.8876x; 1.0512x over previous
"""GAT (2-layer, 8-head) Trainium2 kernel over 8 NeuronCores.

Strategy (edge-cut node sharding), v2:
- Pad N 50000->50176 = 8 shards * 6272. Core c owns nodes [6272c, 6272(c+1)).
- Host: sort edges by dest, bucket into 128-node blocks. Within a block, edges
  are split lo (shard-row < HSPL) / hi, because the batched DMA_GATHER takes
  int16 indices (<=32767) and the full table has 50176 rows. HSPL=2688 is
  128-aligned so the A/B table sections align to 128-row tiles.
- Table-1 row: [Wh in (f,h) head-MINOR order 512 | f_src f16 8 | f_src
  residual 8 | pad -> 640] (1280B, elem %256B). Head-minor order makes the
  per-edge attention scale R = p (*) Wh a DVE TENSOR_TENSOR with an OUTER-dim
  broadcast and innermost stride 1 -> 2x_1P perf mode (the head-major layout
  forces a stride-0 innermost broadcast = 1x).
- One dma_gather per (block, section) (fixed SWDGE overhead ~1us dominates,
  so fewer+bigger gathers), round-robin over 4 queues.
- One-hots (OH: [edge,dst], OHT: [dst,edge]) shipped as fp8e4 (0/1 exact):
  halves their HBM traffic and speeds LDWEIGHTS via FWL.
- leaky-relu on the Scalar engine via Prelu(alpha) (same act table set as Exp
  and Copy -> no table reloads); ELU via hm=Exp(ha) then min/add tensor_scalar
  + max (exp is monotone: e^min(x,0) == min(e^x, 1)).
- AllGathers chunked into 3 pieces per table section and interleaved into the
  producing phase so the collective overlaps compute.
- Softmax needs no segment-max: logits are O(6) so exp never overflows, and
  normalization commutes with the scatter-sum (divide once per node).
"""
import os
import sys
sys.path.insert(0, "/opt/trn_rl_repo")
import numpy as np

import concourse.tile as tile
from concourse import bass, bacc, mybir
from concourse.bass_utils import run_bass_kernel_spmd
from concourse.masks import make_identity

N, E = 50000, 800000
NFEAT, NHID, NHEADS, NCLASS = 512, 64, 8, 64
ALPHA = 0.2
NC = 8
NPAD = 50176
SHARD = NPAD // NC        # 6272
BLK = 128
NBPC = SHARD // BLK       # 49 blocks per core
KT = NFEAT // 128         # 4 k-tiles
DW1 = NFEAT + 16          # 528: Wh | src | src_residual (w1 matmul width)
TW1 = 768                 # table-1 row BYTES: 512 f8e3 Wh | 8+8 f16 fs | pad
DW2 = NCLASS + 2          # 66:  Wh2 | src | src_residual
TW2 = 128                 # table-2 row (256B)
HSPL = 2688               # 21*128; section A rows per shard (8*2688<=32767)
TSPL = HSPL // 128        # 21 tiles in section A

f16d, f32d, i16d = mybir.dt.float16, mybir.dt.float32, mybir.dt.int16
f8d = mybir.dt.float8e4
f8e3 = mybir.dt.float8e3

LAST_EXEC_NS = None
LAST_RESULTS = None
_BUILD_CACHE = {}

def _wrap16(lst):
    """DMA_GATHER index layout: element i at [i%16, i//16], replicated x8."""
    return np.tile(lst.reshape(-1, 16).T, (8, 1))


def _preprocess(row, col):
    order = np.argsort(row, kind="stable")
    row_s = row[order].astype(np.int64)
    col_s = col[order].astype(np.int64)
    counts = np.bincount(row_s // BLK, minlength=NPAD // BLK)
    starts = np.concatenate([[0], np.cumsum(counts)])
    nb = NPAD // BLK
    src_c = col_s // SHARD
    src_r = col_s % SHARD
    in_a = src_r < HSPL
    idx_a = src_c * HSPL + src_r
    idx_b = src_c * (SHARD - HSPL) + (src_r - HSPL)
    nlo = np.zeros(nb, np.int64)
    for b in range(nb):
        nlo[b] = int(in_a[starts[b]:starts[b + 1]].sum())
    nhi = counts - nlo
    cpl = int(((nlo + 127) // 128).max())
    cph = int(((nhi + 127) // 128).max())
    cpe = cpl + cph
    cilo = np.zeros((NC, 128, NBPC * cpl * 8), np.int16)
    cihi = np.zeros((NC, 128, NBPC * cph * 8), np.int16)
    oht = np.zeros((NC, NBPC, 128, cpe * 128), np.float16)
    ohh = np.zeros((NC, NBPC, 128, cpe * 128), np.float16)
    MXC = 4
    NPL = -(-cpl // MXC)
    NP = NPL + -(-cph // MXC)
    gcnt = np.zeros((NC, 1, NBPC * cpe), np.int32)
    iota = np.arange(128)

    def _mark(lst, n, nchunks, warm, gout, gbase):
        """Per 4-chunk piece: count = #idxs to fetch; tail idxs -> -1.
        Warm blocks fetch the full padded piece (initializes pool bufs)."""
        for pi, a in enumerate(range(0, nchunks, MXC)):
            cap = (min(a + MXC, nchunks) - a) * 128
            p0 = a * 128
            valid = min(max(n - p0, 0), cap)
            cnt = cap if warm else min(max(valid, 16), cap)
            lst[p0 + cnt:p0 + cap] = -1
            gout[gbase + pi] = cnt

    for b in range(nb):
        c, bl = divmod(b, NBPC)
        warm = bl < 8
        s, e = starts[b], starts[b + 1]
        rloc = row_s[s:e] - b * BLK
        m = in_a[s:e]
        lo_c, lo_r = idx_a[s:e][m], rloc[m]
        hi_c, hi_r = idx_b[s:e][~m], rloc[~m]
        lst = np.zeros(cpl * 128, np.int16)
        lst[:len(lo_c)] = lo_c
        _mark(lst, len(lo_c), cpl, warm, gcnt[c, 0], bl * NP)
        cilo[c, :, bl * cpl * 8:(bl + 1) * cpl * 8] = _wrap16(lst)
        rlo = np.full(cpl * 128, 200.0, np.float16)
        rlo[:len(lo_r)] = lo_r
        lst2 = np.zeros(cph * 128, np.int16)
        lst2[:len(hi_c)] = hi_c
        _mark(lst2, len(hi_c), cph, warm, gcnt[c, 0], bl * NP + NPL)
        cihi[c, :, bl * cph * 8:(bl + 1) * cph * 8] = _wrap16(lst2)
        rhi = np.full(cph * 128, 200.0, np.float16)
        rhi[:len(hi_r)] = hi_r
        rf = np.concatenate([rlo, rhi])
        oht[c, bl] = (iota[:, None] == rf[None, :]).astype(np.float16)
        rfs = rf.reshape(cpe, 128)
        for cc in range(cpe):
            ohh[c, bl][:, cc * 128:(cc + 1) * 128] = (
                rfs[cc][:, None] == iota[None, :]).astype(np.float16)
    return cilo, cihi, oht, ohh, gcnt, cpl, cph


def _build(cpl, cph):
    key = (cpl, cph, os.environ.get("GAT_FP8_OH", "1"),
           os.environ.get("GAT_PRELU", "1"), os.environ.get("GAT_MAXCH", "4"),
           os.environ.get("GAT_MAXCH2", "4"), os.environ.get("GAT_CSPL", "15"),
           os.environ.get("GAT_EXACT", "1"))
    if key in _BUILD_CACHE:
        return _BUILD_CACHE[key]
    cpe = cpl + cph
    ohd = f8d if int(os.environ.get("GAT_FP8_OH", "1")) else f16d
    use_prelu = bool(int(os.environ.get("GAT_PRELU", "1")))
    nc = bacc.Bacc("TRN2", target_bir_lowering=False, debug=False,
                   enable_asserts=True, num_devices=NC, num_swdge_queues=4)
    xt = nc.dram_tensor("xt", [NBPC, 128, KT * 128], f16d, kind="ExternalInput")
    w1 = nc.dram_tensor("w1", [KT * 128, DW1], f16d, kind="ExternalInput")
    w2 = nc.dram_tensor("w2", [KT * 128, DW2], f16d, kind="ExternalInput")
    cilo = nc.dram_tensor("cilo", [128, NBPC * cpl * 8], i16d, kind="ExternalInput")
    cihi = nc.dram_tensor("cihi", [128, NBPC * cph * 8], i16d, kind="ExternalInput")
    ohtT = nc.dram_tensor("ohtT", [NBPC, 128, cpe * 128], ohd,
                          kind="ExternalInput")
    ohhT = nc.dram_tensor("ohhT", [NBPC, 128, cpe * 128], ohd,
                          kind="ExternalInput")
    gcnt = nc.dram_tensor("gcnt", [1, NBPC * (cpl + cph)], mybir.dt.int32,
                          kind="ExternalInput")
    out = nc.dram_tensor("out", [SHARD, NCLASS], f32d, kind="ExternalOutput")

    AF, ALU = mybir.ActivationFunctionType, mybir.AluOpType
    HSPB = SHARD - HSPL

    qctr = [0]
    # SWDGE dma_gather hangs above 512 idxs/instruction (at 1280B elems)
    maxch = int(os.environ.get("GAT_MAXCH", "4"))
    maxch2 = int(os.environ.get("GAT_MAXCH2", "4"))
    CSPL = min(cpl + cph, int(os.environ.get("GAT_CSPL", "15")))
    exact = bool(int(os.environ.get("GAT_EXACT", "1")))
    assert maxch == maxch2 == 4, "gcnt piece layout assumes 4-chunk pieces"
    NPL = -(-cpl // maxch)
    NP = NPL + -(-cph // maxch)

    greg = [None]   # gather-count register, set inside the TileContext
    gcnt_t = [None]  # resident [1, NBPC*NPIECES] int32 per-piece counts

    def _gather(dst, table, idx_t, icol0, nch, elem, mx, ccol):
        a = 0
        pi = 0
        while a < nch:
            b = min(a + mx, nch)
            if greg[0] is not None:
                nc.reg_load(greg[0], gcnt_t[0][0:1, ccol + pi:ccol + pi + 1])
                nreg = greg[0]
            else:
                nreg = (b - a) * 128
            nc.gpsimd.dma_gather(
                out_ap=dst[:, a * elem:b * elem]
                    .rearrange("p (c e) -> p c e", e=elem),
                in_ap=table,
                idxs_ap=idx_t[:, icol0 + a * 8:icol0 + b * 8],
                num_idxs=(b - a) * 128, num_idxs_reg=nreg,
                elem_size=elem, queue_num=qctr[0] % 4)
            qctr[0] += 1
            a = b
            pi += 1

    def gather_sect(dst, table, idx_t, icol0, nch, elem, ccol):
        _gather(dst, table, idx_t, icol0, nch, elem, maxch, ccol)

    def gather_sect2(dst, table, idx_t, icol0, nch, elem, ccol):
        _gather(dst, table, idx_t, icol0, nch, elem, maxch2, ccol)

    with tile.TileContext(nc) as tc:
        with tc.tile_pool(name="res", bufs=1) as res, \
             tc.tile_pool(name="dram", bufs=1, space="DRAM") as drp, \
             nc.gpsimd.register(name="gcr") as gcr:
            if exact:
                greg[0] = gcr
            # shard tables; section-A collective issues mid-phase (a Shared
            # tensor may only be written by ONE instruction, so chunking
            # finer than per-section is not possible)
            tab1sA = drp.tile([HSPL, TW1], f8e3)
            tab1sB = drp.tile([HSPB, TW1], f8e3)
            tab2sA = drp.tile([HSPL, TW2], f16d)
            tab2sB = drp.tile([HSPB, TW2], f16d)
            tab1a = drp.tile([NC * HSPL, TW1], f8e3, addr_space="Shared")
            tab1b = drp.tile([NC * HSPB, TW1], f8e3, addr_space="Shared")
            tab2a = drp.tile([NC * HSPL, TW2], f16d, addr_space="Shared")
            tab2b = drp.tile([NC * HSPB, TW2], f16d, addr_space="Shared")

            def ag_sect(src, dst):
                nc.gpsimd.collective_compute(
                    "AllGather", ALU.bypass, replica_groups=[list(range(NC))],
                    ins=[src.opt()],
                    outs=[dst[:].rearrange("(c r) d -> c r d", c=NC)])

            w1_t = res.tile([128, KT * DW1], f16d)
            w2_t = res.tile([128, KT * DW2], f16d)
            for k in range(KT):
                nc.sync.dma_start(out=w1_t[:, k * DW1:(k + 1) * DW1],
                                  in_=w1[k * 128:(k + 1) * 128, :])
                nc.sync.dma_start(out=w2_t[:, k * DW2:(k + 1) * DW2],
                                  in_=w2[k * 128:(k + 1) * 128, :])
            cilo_t = res.tile([128, NBPC * cpl * 8], i16d)
            cihi_t = res.tile([128, NBPC * cph * 8], i16d)
            nc.sync.dma_start(out=cilo_t[:], in_=cilo[:, :])
            nc.sync.dma_start(out=cihi_t[:], in_=cihi[:, :])
            ident = res.tile([128, 128], f16d)
            make_identity(nc, ident[:])
            gct = res.tile([1, NBPC * (cpl + cph)], mybir.dt.int32)
            nc.sync.dma_start(out=gct[:], in_=gcnt[0:1, :])
            gcnt_t[0] = gct
            fd_sb = res.tile([128, NBPC * 8], f16d)
            fd2_sb = res.tile([128, NBPC], f16d)

            # ---------------- Phase A ----------------
            with nc.named_scope("phaseA"), \
                 tc.tile_pool(name="pa", bufs=5) as pa, \
                 tc.tile_pool(name="ppa", bufs=2, space="PSUM") as ppa:
                def issue_a(nt):
                    xk4 = pa.tile([128, KT * 128], f16d, tag="xk4")
                    nc.sync.dma_start(out=xk4[:], in_=xt[nt, :, :])
                    return xk4
                PFA = 3
                penda = [issue_a(t) for t in range(PFA)]
                for nt in range(NBPC):
                    psA = ppa.tile([128, 512], f32d, tag="psA")
                    psB = ppa.tile([128, 16], f32d, tag="psB")
                    xk4 = penda.pop(0)
                    if nt + PFA < NBPC:
                        penda.append(issue_a(nt + PFA))
                    for k in range(KT):
                        nc.tensor.matmul(out=psA[:],
                                         lhsT=xk4[:, k * 128:(k + 1) * 128],
                                         rhs=w1_t[:, k * DW1:k * DW1 + 512],
                                         start=(k == 0), stop=(k == KT - 1))
                        nc.tensor.matmul(out=psB[:],
                                         lhsT=xk4[:, k * 128:(k + 1) * 128],
                                         rhs=w1_t[:, k * DW1 + 512:(k + 1) * DW1],
                                         start=(k == 0), stop=(k == KT - 1))
                    whf = pa.tile([128, TW1], f8e3, tag="whf")
                    wh16 = whf[:].bitcast(f16d)      # [128, TW1//2] f16 view
                    nc.vector.tensor_copy(out=whf[:, :512], in_=psA[:])
                    nc.scalar.activation(out=wh16[:, 256:264], in_=psB[:, 8:16],
                                         func=AF.Copy)
                    nc.vector.tensor_tensor(out=wh16[:, 264:272],
                                            in0=psB[:, 8:16],
                                            in1=wh16[:, 256:264],
                                            op=ALU.subtract)
                    nc.vector.tensor_copy(out=fd_sb[:, nt * 8:(nt + 1) * 8],
                                          in_=psB[:, 0:8])
                    if nt < TSPL:
                        r0 = nt * 128
                        nc.scalar.dma_start(out=tab1sA[r0:r0 + 128, 0:544],
                                            in_=whf[:, 0:544])
                    else:
                        r0 = (nt - TSPL) * 128
                        nc.scalar.dma_start(out=tab1sB[r0:r0 + 128, 0:544],
                                            in_=whf[:, 0:544])
                    if nt == TSPL - 1:
                        with nc.named_scope("ag1"):
                            ag_sect(tab1sA, tab1a)
                    if nt == NBPC - 1:
                        with nc.named_scope("ag1"):
                            ag_sect(tab1sB, tab1b)

            # ---------------- Phase B ----------------
            # 2-stage software pipeline: stage1(bl) = logits+R (DVE/ACT +
            # small PE), stage2(bl) = scatter/elu/layer-2 (big PE + DVE/ACT).
            # Interleaving stage1(i) with stage2(i-1) keeps every in-order
            # engine queue supplied with ready work (no head-of-line stalls).
            with nc.named_scope("phaseB"), \
                 tc.tile_pool(name="pb", bufs=2) as pb, \
                 tc.tile_pool(name="pr", bufs=3) as pr, \
                 tc.tile_pool(name="pgg", bufs=6) as pgg, \
                 tc.tile_pool(name="pgo", bufs=6) as pgo, \
                 tc.tile_pool(name="ppb", bufs=2, space="PSUM") as ppb, \
                 tc.tile_pool(name="ppf", bufs=3, space="PSUM") as ppf, \
                 tc.tile_pool(name="ppt", bufs=2, space="PSUM") as ppt:
                def issue_b(bl):
                    G = pgg.tile([128, cpe * TW1], f8e3, tag="G")
                    OH = pgo.tile([128, cpe * 128], ohd, tag="OH")
                    OHT = pgo.tile([128, cpe * 128], ohd, tag="OHT")
                    nc.sync.dma_start(out=OHT[:], in_=ohtT[bl, :, :])
                    nc.sync.dma_start(out=OH[:], in_=ohhT[bl, :, :])
                    gather_sect(G[:, :cpl * TW1], tab1a[:, :],
                                cilo_t, bl * cpl * 8, cpl, TW1, bl * NP)
                    gather_sect(G[:, cpl * TW1:], tab1b[:, :],
                                cihi_t, bl * cph * 8, cph, TW1, bl * NP + NPL)
                    return G, OH, OHT

                PF = 3
                pend = [issue_b(b) for b in range(PF)]

                def stage1(bl):
                    G, OH, OHT = pend.pop(0)
                    if bl + PF < NBPC:
                        pend.append(issue_b(bl + PF))
                    aux = ppf.tile([128, 8 + cpe * 8 + DW2], f32d, tag="aux")
                    pfd = aux[:, 8:8 + cpe * 8]
                    for c in range(cpe):
                        nc.tensor.matmul(out=pfd[:, c * 8:(c + 1) * 8],
                                         lhsT=OHT[:, c * 128:(c + 1) * 128],
                                         rhs=fd_sb[:, bl * 8:(bl + 1) * 8],
                                         start=True, stop=True)
                    e1 = pb.tile([128, cpe * 8], f32d, tag="e1")
                    p16 = pr.tile([128, cpe * 8], f16d, tag="p16")
                    Gr = G[:].rearrange("p (c d) -> p c d", d=TW1)
                    Gf = G[:].bitcast(f16d).rearrange("p (c d) -> p c d",
                                                      d=TW1 // 2)
                    nc.vector.tensor_tensor(
                        out=e1[:].rearrange("p (c f) -> p c f", c=cpe),
                        in0=Gf[:, :, 256:264],
                        in1=Gf[:, :, 264:272], op=ALU.add)
                    nc.vector.tensor_tensor(out=e1[:], in0=e1[:], in1=pfd,
                                            op=ALU.add)
                    if use_prelu:
                        nc.scalar.activation(out=e1[:], in_=e1[:],
                                             func=AF.Prelu, alpha=ALPHA)
                    else:
                        lr = pb.tile([128, cpe * 8], f32d, tag="lr")
                        nc.vector.tensor_scalar_mul(lr[:], e1[:], ALPHA)
                        nc.vector.tensor_tensor(out=e1[:], in0=e1[:],
                                                in1=lr[:], op=ALU.max)
                    nc.scalar.activation(out=p16[:], in_=e1[:], func=AF.Exp)
                    # R[p, c, f, h] = Wh[p, c, f, h] * p16[p, c, h].
                    # Wh is fp8 (1-byte) which forbids DVE 2x mode, so split:
                    # chunks [0, CSPL) are upcast to f16 on the Scalar engine
                    # then multiplied at 2x on DVE; chunks [CSPL, cpe) are
                    # multiplied directly from fp8 at 1x. CSPL balances the
                    # two engines.
                    R = pr.tile([128, cpe * 512], f16d, tag="R")
                    if CSPL > 0:
                        nc.scalar.activation(
                            out=R[:, :CSPL * 512], in_=Gr[:, 0:CSPL, 0:512],
                            func=AF.Copy)
                        nc.vector.tensor_tensor(
                            out=R[:, :CSPL * 512]
                                .rearrange("p (c f h) -> p c f h", f=64, h=8),
                            in0=R[:, :CSPL * 512]
                                .rearrange("p (c f h) -> p c f h", f=64, h=8),
                            in1=p16[:, :CSPL * 8]
                                .rearrange("p (c o h) -> p c o h", o=1, h=8)
                                .to_broadcast([128, CSPL, 64, 8]),
                            op=ALU.mult)
                    if CSPL < cpe:
                        nc.vector.tensor_tensor(
                            out=R[:, CSPL * 512:]
                                .rearrange("p (c f h) -> p c f h", f=64, h=8),
                            in0=Gr[:, CSPL:, 0:512]
                                .rearrange("p c (f h) -> p c f h", f=64),
                            in1=p16[:, CSPL * 8:]
                                .rearrange("p (c o h) -> p c o h", o=1, h=8)
                                .to_broadcast([128, cpe - CSPL, 64, 8]),
                            op=ALU.mult)
                    return (bl, OH, aux, R, p16)

                def stage2(st):
                    bl, OH, aux, R, p16 = st
                    pden = aux[:, 0:8]
                    ps2 = aux[:, 8 + cpe * 8:8 + cpe * 8 + DW2]
                    pnum = ppb.tile([128, 512], f32d, tag="pnum")
                    for i in range(cpe):
                        nc.tensor.matmul(out=pnum[:],
                                         lhsT=OH[:, i * 128:(i + 1) * 128],
                                         rhs=R[:, i * 512:(i + 1) * 512],
                                         start=(i == 0), stop=(i == cpe - 1))
                        nc.tensor.matmul(out=pden,
                                         lhsT=OH[:, i * 128:(i + 1) * 128],
                                         rhs=p16[:, i * 8:(i + 1) * 8],
                                         start=(i == 0), stop=(i == cpe - 1))
                    dcl = pb.tile([128, 8], f32d, tag="dcl")
                    nc.vector.tensor_scalar_max(dcl[:], pden, 1e-30)
                    nc.vector.reciprocal(out=dcl[:], in_=dcl[:])
                    ha = pb.tile([128, 512], f16d, tag="ha")
                    nc.vector.tensor_tensor(
                        out=ha[:].rearrange("p (f h) -> p f h", f=64),
                        in0=pnum[:].rearrange("p (f h) -> p f h", f=64),
                        in1=dcl[:].rearrange("p (o h) -> p o h", o=1)
                            .to_broadcast([128, 64, 8]),
                        op=ALU.mult)
                    # elu: h16 = max(ha, min(e^ha, 1) - 1)  (exp monotone)
                    hm = pb.tile([128, 512], f16d, tag="hm")
                    nc.scalar.activation(out=hm[:], in_=ha[:], func=AF.Exp)
                    nc.vector.tensor_scalar(out=hm[:], in0=hm[:],
                                            scalar1=1.0, scalar2=-1.0,
                                            op0=ALU.min, op1=ALU.add)
                    h16 = pb.tile([128, 512], f16d, tag="h16")
                    nc.vector.tensor_tensor(out=h16[:], in0=hm[:], in1=ha[:],
                                            op=ALU.max)
                    for k in range(KT):
                        pt = ppt.tile([128, 128], f16d, tag="pt")
                        nc.tensor.transpose(out=pt[:],
                                            in_=h16[:, k * 128:(k + 1) * 128],
                                            identity=ident[:])
                        ht = pb.tile([128, 128], f16d, tag="ht")
                        if k % 2 == 0:
                            nc.vector.tensor_copy(out=ht[:], in_=pt[:])
                        else:
                            nc.scalar.activation(out=ht[:], in_=pt[:],
                                                 func=AF.Copy)
                        nc.tensor.matmul(out=ps2, lhsT=ht[:],
                                         rhs=w2_t[:, k * DW2:(k + 1) * DW2],
                                         start=(k == 0), stop=(k == KT - 1))
                    t2 = pb.tile([128, DW2], f16d, tag="t2")
                    nc.scalar.activation(out=t2[:, 0:64], in_=ps2[:, 0:64],
                                         func=AF.Copy)
                    nc.scalar.activation(out=t2[:, 64:65], in_=ps2[:, 65:66],
                                         func=AF.Copy)
                    nc.vector.tensor_tensor(out=t2[:, 65:66], in0=ps2[:, 65:66],
                                            in1=t2[:, 64:65], op=ALU.subtract)
                    nc.scalar.activation(out=fd2_sb[:, bl:bl + 1],
                                         in_=ps2[:, 64:65], func=AF.Copy)
                    if bl < TSPL:
                        r0 = bl * 128
                        nc.scalar.dma_start(out=tab2sA[r0:r0 + 128, 0:DW2],
                                            in_=t2[:])
                    else:
                        r0 = (bl - TSPL) * 128
                        nc.scalar.dma_start(out=tab2sB[r0:r0 + 128, 0:DW2],
                                            in_=t2[:])
                    if bl == TSPL - 1:
                        with nc.named_scope("ag2"):
                            ag_sect(tab2sA, tab2a)
                    if bl == NBPC - 1:
                        with nc.named_scope("ag2"):
                            ag_sect(tab2sB, tab2b)

                live = []
                for bl in range(NBPC):
                    live.append(stage1(bl))
                    if len(live) > 1:
                        stage2(live.pop(0))
                stage2(live.pop(0))

            # ---------------- Phase C ----------------
            with nc.named_scope("phaseC"), \
                 tc.tile_pool(name="pc", bufs=2) as pc, \
                 tc.tile_pool(name="pr2", bufs=3) as pr2, \
                 tc.tile_pool(name="pg2", bufs=6) as pg2, \
                 tc.tile_pool(name="pgo2", bufs=6) as pgo2, \
                 tc.tile_pool(name="ppc", bufs=3, space="PSUM") as ppc:
                def issue_c(bl):
                    G2 = pg2.tile([128, cpe * TW2], f16d, tag="G2")
                    OH2 = pgo2.tile([128, cpe * 128], ohd, tag="OH2")
                    OH2T = pgo2.tile([128, cpe * 128], ohd, tag="OH2T")
                    nc.sync.dma_start(out=OH2[:], in_=ohhT[bl, :, :])
                    nc.sync.dma_start(out=OH2T[:], in_=ohtT[bl, :, :])
                    gather_sect2(G2[:, :cpl * TW2], tab2a[:, :],
                                 cilo_t, bl * cpl * 8, cpl, TW2, bl * NP)
                    gather_sect2(G2[:, cpl * TW2:], tab2b[:, :],
                                 cihi_t, bl * cph * 8, cph, TW2,
                                 bl * NP + NPL)
                    return G2, OH2, OH2T

                PF = 3
                pend2 = [issue_c(b) for b in range(PF)]

                def stage1c(bl):
                    G2, OH2, OH2T = pend2.pop(0)
                    if bl + PF < NBPC:
                        pend2.append(issue_c(bl + PF))
                    pfd2 = ppc.tile([128, cpe], f32d, tag="pfd2")
                    for c in range(cpe):
                        nc.tensor.matmul(out=pfd2[:, c:c + 1],
                                         lhsT=OH2T[:, c * 128:(c + 1) * 128],
                                         rhs=fd2_sb[:, bl:bl + 1],
                                         start=True, stop=True)
                    e2 = pc.tile([128, cpe], f32d, tag="e2")
                    p2 = pr2.tile([128, cpe], f16d, tag="p2")
                    G2r = G2[:].rearrange("p (c d) -> p c d", d=TW2)
                    nc.vector.tensor_tensor(
                        out=e2[:].rearrange("p (c o) -> p c o", o=1),
                        in0=G2r[:, :, 64:65],
                        in1=G2r[:, :, 65:66], op=ALU.add)
                    nc.vector.tensor_tensor(
                        out=e2[:], in0=e2[:], in1=pfd2[:], op=ALU.add)
                    if use_prelu:
                        nc.scalar.activation(out=e2[:], in_=e2[:],
                                             func=AF.Prelu, alpha=ALPHA)
                    else:
                        lr2 = pc.tile([128, cpe], f32d, tag="lr2")
                        nc.vector.tensor_scalar_mul(lr2[:], e2[:], ALPHA)
                        nc.vector.tensor_tensor(out=e2[:], in0=e2[:],
                                                in1=lr2[:], op=ALU.max)
                    nc.scalar.activation(out=p2[:], in_=e2[:], func=AF.Exp)
                    R2 = pr2.tile([128, cpe * 65], f16d, tag="R2")
                    R2r = R2[:].rearrange("p (c d) -> p c d", d=65)
                    nc.vector.tensor_tensor(
                        out=R2r[:, :, 0:64],
                        in0=G2r[:, :, 0:64],
                        in1=p2[:].to_broadcast([128, cpe, 64]),
                        op=ALU.mult)
                    nc.vector.tensor_copy(
                        out=R2r[:, :, 64:65],
                        in_=p2[:].rearrange("p (c o) -> p c o", o=1))
                    return (bl, OH2, R2)

                def stage2c(st):
                    bl, OH2, R2 = st
                    rows = slice(bl * 128, (bl + 1) * 128)
                    ps3 = ppc.tile([128, 65], f32d, tag="ps3")
                    for i in range(cpe):
                        nc.tensor.matmul(out=ps3[:],
                                         lhsT=OH2[:, i * 128:(i + 1) * 128],
                                         rhs=R2[:, i * 65:(i + 1) * 65],
                                         start=(i == 0), stop=(i == cpe - 1))
                    d2c = pc.tile([128, 1], f32d, tag="d2c")
                    nc.vector.tensor_scalar_max(d2c[:], ps3[:, 64:65], 1e-30)
                    nc.vector.reciprocal(out=d2c[:], in_=d2c[:])
                    o = pc.tile([128, 64], f32d, tag="o")
                    nc.vector.tensor_tensor(
                        out=o[:].rearrange("p (c f) -> p c f", c=1),
                        in0=ps3[:, 0:64].rearrange("p (c f) -> p c f", c=1),
                        in1=d2c[:].to_broadcast([128, 1, 64]),
                        op=ALU.mult)
                    nc.scalar.dma_start(out=out[rows, :], in_=o[:])

                live2 = []
                for bl in range(NBPC):
                    live2.append(stage1c(bl))
                    if len(live2) > 1:
                        stage2c(live2.pop(0))
                stage2c(live2.pop(0))

    nc.compile()
    _BUILD_CACHE[key] = nc
    return nc


def kernel(**inputs):
    global LAST_EXEC_NS, LAST_RESULTS
    x = inputs["x"].astype(np.float32)
    row = inputs["row"].astype(np.int64)
    col = inputs["col"].astype(np.int64)
    W, a = inputs["W"].astype(np.float32), inputs["a"].astype(np.float32)
    W_out = inputs["W_out"].astype(np.float32)
    a_out = inputs["a_out"].astype(np.float32)

    cilo, cihi, oht, ohh, gcnt, cpl, cph = _preprocess(row, col)

    # head-MINOR (f,h) feature order for layer-1 Wh and layer-2 rows
    W_cat = np.stack([W[h] for h in range(NHEADS)], axis=-1)  # [in, f, h]
    W_cat = W_cat.reshape(NFEAT, NHID * NHEADS)
    WA_dst = np.stack([W[h] @ a[h, :NHID] for h in range(NHEADS)], 1)
    WA_src = np.stack([W[h] @ a[h, NHID:] for h in range(NHEADS)], 1)
    w1_np = np.concatenate([W_cat, WA_dst, WA_src], 1).astype(np.float16)
    w2full = np.concatenate([W_out, (W_out @ a_out[:NCLASS])[:, None],
                             (W_out @ a_out[NCLASS:])[:, None]], 1)
    idx = np.arange(NHID * NHEADS)
    perm = (idx % NHEADS) * NHID + idx // NHEADS   # (f,h) -> h*64+f
    w2_np = w2full[perm, :].astype(np.float16)

    x_pad = np.zeros((NPAD, NFEAT), np.float16)
    x_pad[:N] = x

    nc = _build(cpl, cph)

    fp8 = bool(int(os.environ.get("GAT_FP8_OH", "1")))
    ohdt = mybir.dt.np(f8d) if fp8 else np.float16
    in_maps = []
    for c in range(NC):
        xs = x_pad[c * SHARD:(c + 1) * SHARD]            # [6272, 512]
        xt = (xs.reshape(NBPC, 128, KT, 128)             # [nt, n, k, f]
                .transpose(0, 3, 2, 1)                   # [nt, f, k, n]
                .reshape(NBPC, 128, KT * 128)).copy()
        in_maps.append({"xt": xt, "w1": w1_np, "w2": w2_np,
                        "cilo": cilo[c], "cihi": cihi[c],
                        "gcnt": gcnt[c],
                        "ohtT": oht[c].astype(ohdt),
                        "ohhT": ohh[c].astype(ohdt)})

    trace = bool(int(os.environ.get("GAT_TRACE", "0")))
    res = run_bass_kernel_spmd(nc, in_maps, list(range(NC)), trace=trace,
                               trace_cores=list(range(NC)) if trace else None)
    LAST_EXEC_NS = res.exec_time_ns
    LAST_RESULTS = res
    outs = [res.results[c]["out"] for c in range(NC)]
    return np.concatenate(outs, 0)[:N].astype(np.float32)


# revision 33
# speedup vs baseline: 17948.7744x; 1.0145x over previous
"""GAT (2-layer, 8-head) Trainium2 kernel over 8 NeuronCores.

Strategy (edge-cut node sharding), v2:
- Pad N 50000->50176 = 8 shards * 6272. Core c owns nodes [6272c, 6272(c+1)).
- Host: sort edges by dest, bucket into 128-node blocks. Within a block, edges
  are split lo (shard-row < HSPL) / hi, because the batched DMA_GATHER takes
  int16 indices (<=32767) and the full table has 50176 rows. HSPL=2688 is
  128-aligned so the A/B table sections align to 128-row tiles.
- Table-1 row: [Wh in (f,h) head-MINOR order 512 | f_src f16 8 | f_src
  residual 8 | pad -> 640] (1280B, elem %256B). Head-minor order makes the
  per-edge attention scale R = p (*) Wh a DVE TENSOR_TENSOR with an OUTER-dim
  broadcast and innermost stride 1 -> 2x_1P perf mode (the head-major layout
  forces a stride-0 innermost broadcast = 1x).
- One dma_gather per (block, section) (fixed SWDGE overhead ~1us dominates,
  so fewer+bigger gathers), round-robin over 4 queues.
- One-hots (OH: [edge,dst], OHT: [dst,edge]) shipped as fp8e4 (0/1 exact):
  halves their HBM traffic and speeds LDWEIGHTS via FWL.
- leaky-relu on the Scalar engine via Prelu(alpha) (same act table set as Exp
  and Copy -> no table reloads); ELU via hm=Exp(ha) then min/add tensor_scalar
  + max (exp is monotone: e^min(x,0) == min(e^x, 1)).
- AllGathers chunked into 3 pieces per table section and interleaved into the
  producing phase so the collective overlaps compute.
- Softmax needs no segment-max: logits are O(6) so exp never overflows, and
  normalization commutes with the scatter-sum (divide once per node).
"""
import contextlib
import os
import sys
sys.path.insert(0, "/opt/trn_rl_repo")
import numpy as np

import concourse.tile as tile
from concourse import bass, bacc, mybir
from concourse.bass_utils import run_bass_kernel_spmd
from concourse.masks import make_identity

N, E = 50000, 800000
NFEAT, NHID, NHEADS, NCLASS = 512, 64, 8, 64
ALPHA = 0.2
NC = 8
NPAD = 50176
SHARD = NPAD // NC        # 6272
BLK = 128
NBPC = SHARD // BLK       # 49 blocks per core
KT = NFEAT // 128         # 4 k-tiles
DW1 = NFEAT + 16          # 528: Wh | src | src_residual (w1 matmul width)
TW1 = 768                 # table-1 row BYTES: 512 f8e3 Wh | 8+8 f16 fs | pad
DW2 = NCLASS + 2          # 66:  Wh2 | src | src_residual
TW2 = 128                 # table-2 row (256B)
HSPL = 2688               # 21*128; section A rows per shard (8*2688<=32767)
TSPL = HSPL // 128        # 21 tiles in section A

f16d, f32d, i16d = mybir.dt.float16, mybir.dt.float32, mybir.dt.int16
f8d = mybir.dt.float8e4
f8e3 = mybir.dt.float8e3

LAST_EXEC_NS = None
LAST_RESULTS = None
_BUILD_CACHE = {}

def _wrap16(lst):
    """DMA_GATHER index layout: element i at [i%16, i//16], replicated x8."""
    return np.tile(lst.reshape(-1, 16).T, (8, 1))


def _preprocess(row, col):
    order = np.argsort(row, kind="stable")
    row_s = row[order].astype(np.int64)
    col_s = col[order].astype(np.int64)
    counts = np.bincount(row_s // BLK, minlength=NPAD // BLK)
    starts = np.concatenate([[0], np.cumsum(counts)])
    nb = NPAD // BLK
    src_c = col_s // SHARD
    src_r = col_s % SHARD
    in_a = src_r < HSPL
    idx_a = src_c * HSPL + src_r
    idx_b = src_c * (SHARD - HSPL) + (src_r - HSPL)
    nlo = np.zeros(nb, np.int64)
    for b in range(nb):
        nlo[b] = int(in_a[starts[b]:starts[b + 1]].sum())
    nhi = counts - nlo
    cpl = int(((nlo + 127) // 128).max())
    cph = int(((nhi + 127) // 128).max())
    cpe = cpl + cph
    cilo = np.zeros((NC, 128, NBPC * cpl * 8), np.int16)
    cihi = np.zeros((NC, 128, NBPC * cph * 8), np.int16)
    oht = np.zeros((NC, NBPC, 128, cpe * 128), np.float16)
    ohh = np.zeros((NC, NBPC, 128, cpe * 128), np.float16)
    MXC = 4
    NPL = -(-cpl // MXC)
    NP = NPL + -(-cph // MXC)
    gcnt = np.zeros((NC, 1, NBPC * cpe), np.int32)
    iota = np.arange(128)

    def _mark(lst, n, nchunks, warm, gout, gbase):
        """Per 4-chunk piece: count = #idxs to fetch; tail idxs -> -1.
        Warm blocks fetch the full padded piece (initializes pool bufs)."""
        for pi, a in enumerate(range(0, nchunks, MXC)):
            cap = (min(a + MXC, nchunks) - a) * 128
            p0 = a * 128
            valid = min(max(n - p0, 0), cap)
            cnt = cap if warm else min(max(valid, 16), cap)
            lst[p0 + cnt:p0 + cap] = -1
            gout[gbase + pi] = cnt

    for b in range(nb):
        c, bl = divmod(b, NBPC)
        warm = bl < 8
        s, e = starts[b], starts[b + 1]
        rloc = row_s[s:e] - b * BLK
        m = in_a[s:e]
        lo_c, lo_r = idx_a[s:e][m], rloc[m]
        hi_c, hi_r = idx_b[s:e][~m], rloc[~m]
        lst = np.zeros(cpl * 128, np.int16)
        lst[:len(lo_c)] = lo_c
        _mark(lst, len(lo_c), cpl, warm, gcnt[c, 0], bl * NP)
        cilo[c, :, bl * cpl * 8:(bl + 1) * cpl * 8] = _wrap16(lst)
        rlo = np.full(cpl * 128, 200.0, np.float16)
        rlo[:len(lo_r)] = lo_r
        lst2 = np.zeros(cph * 128, np.int16)
        lst2[:len(hi_c)] = hi_c
        _mark(lst2, len(hi_c), cph, warm, gcnt[c, 0], bl * NP + NPL)
        cihi[c, :, bl * cph * 8:(bl + 1) * cph * 8] = _wrap16(lst2)
        rhi = np.full(cph * 128, 200.0, np.float16)
        rhi[:len(hi_r)] = hi_r
        rf = np.concatenate([rlo, rhi])
        oht[c, bl] = (iota[:, None] == rf[None, :]).astype(np.float16)
        rfs = rf.reshape(cpe, 128)
        for cc in range(cpe):
            ohh[c, bl][:, cc * 128:(cc + 1) * 128] = (
                rfs[cc][:, None] == iota[None, :]).astype(np.float16)
    return cilo, cihi, oht, ohh, gcnt, cpl, cph


def _build(cpl, cph):
    key = (cpl, cph, os.environ.get("GAT_FP8_OH", "1"),
           os.environ.get("GAT_PRELU", "1"), os.environ.get("GAT_MAXCH", "4"),
           os.environ.get("GAT_MAXCH2", "4"), os.environ.get("GAT_CSPL", "12"),
           os.environ.get("GAT_EXACT", "1"))
    if key in _BUILD_CACHE:
        return _BUILD_CACHE[key]
    cpe = cpl + cph
    ohd = f8d if int(os.environ.get("GAT_FP8_OH", "1")) else f16d
    use_prelu = bool(int(os.environ.get("GAT_PRELU", "1")))
    nc = bacc.Bacc("TRN2", target_bir_lowering=False, debug=False,
                   enable_asserts=True, num_devices=NC, num_swdge_queues=4)
    xt = nc.dram_tensor("xt", [NBPC, 128, KT * 128], f16d, kind="ExternalInput")
    w1 = nc.dram_tensor("w1", [KT * 128, DW1], f16d, kind="ExternalInput")
    w2 = nc.dram_tensor("w2", [KT * 128, DW2], f16d, kind="ExternalInput")
    cilo = nc.dram_tensor("cilo", [128, NBPC * cpl * 8], i16d, kind="ExternalInput")
    cihi = nc.dram_tensor("cihi", [128, NBPC * cph * 8], i16d, kind="ExternalInput")
    ohtT = nc.dram_tensor("ohtT", [NBPC, 128, cpe * 128], ohd,
                          kind="ExternalInput")
    ohhT = nc.dram_tensor("ohhT", [NBPC, 128, cpe * 128], ohd,
                          kind="ExternalInput")
    gcnt = nc.dram_tensor("gcnt", [1, NBPC * (cpl + cph)], mybir.dt.int32,
                          kind="ExternalInput")
    out = nc.dram_tensor("out", [SHARD, NCLASS], f32d, kind="ExternalOutput")

    AF, ALU = mybir.ActivationFunctionType, mybir.AluOpType
    HSPB = SHARD - HSPL

    qctr = [0]
    # SWDGE dma_gather hangs above 512 idxs/instruction (at 1280B elems)
    maxch = int(os.environ.get("GAT_MAXCH", "4"))
    maxch2 = int(os.environ.get("GAT_MAXCH2", "4"))
    CSPL = min(cpl + cph, int(os.environ.get("GAT_CSPL", "12")))
    exact = bool(int(os.environ.get("GAT_EXACT", "1")))
    assert maxch == maxch2 == 4, "gcnt piece layout assumes 4-chunk pieces"
    NPL = -(-cpl // maxch)
    NP = NPL + -(-cph // maxch)

    gregs = []       # per-piece gather-count registers (one block's worth)
    gcnt_t = [None]  # resident [1, NBPC*NP] int32 per-piece counts

    def load_counts(bl):
        """One reg_load fills all NP piece-count registers for block bl."""
        if gregs:
            nc.reg_load(gregs, gcnt_t[0][0:1, bl * NP:bl * NP + NP])

    def _gather(dst, table, idx_t, icol0, nch, elem, mx, p0):
        a = 0
        pi = 0
        while a < nch:
            b = min(a + mx, nch)
            nreg = gregs[p0 + pi] if gregs else (b - a) * 128
            nc.gpsimd.dma_gather(
                out_ap=dst[:, a * elem:b * elem]
                    .rearrange("p (c e) -> p c e", e=elem),
                in_ap=table,
                idxs_ap=idx_t[:, icol0 + a * 8:icol0 + b * 8],
                num_idxs=(b - a) * 128, num_idxs_reg=nreg,
                elem_size=elem, queue_num=qctr[0] % 4)
            qctr[0] += 1
            a = b
            pi += 1

    def gather_sect(dst, table, idx_t, icol0, nch, elem, p0):
        _gather(dst, table, idx_t, icol0, nch, elem, maxch, p0)

    def gather_sect2(dst, table, idx_t, icol0, nch, elem, p0):
        _gather(dst, table, idx_t, icol0, nch, elem, maxch2, p0)

    with tile.TileContext(nc) as tc:
        with contextlib.ExitStack() as est, \
             tc.tile_pool(name="res", bufs=1) as res, \
             tc.tile_pool(name="dram", bufs=1, space="DRAM") as drp:
            if exact:
                for i in range(NP):
                    gregs.append(est.enter_context(
                        nc.gpsimd.register(name=f"gcr{i}")))
            # shard tables; section-A collective issues mid-phase (a Shared
            # tensor may only be written by ONE instruction, so chunking
            # finer than per-section is not possible)
            tab1sA = drp.tile([HSPL, TW1], f8e3)
            tab1sB = drp.tile([HSPB, TW1], f8e3)
            tab2sA = drp.tile([HSPL, TW2], f16d)
            tab2sB = drp.tile([HSPB, TW2], f16d)
            tab1a = drp.tile([NC * HSPL, TW1], f8e3, addr_space="Shared")
            tab1b = drp.tile([NC * HSPB, TW1], f8e3, addr_space="Shared")
            tab2a = drp.tile([NC * HSPL, TW2], f16d, addr_space="Shared")
            tab2b = drp.tile([NC * HSPB, TW2], f16d, addr_space="Shared")

            def ag_sect(src, dst):
                nc.gpsimd.collective_compute(
                    "AllGather", ALU.bypass, replica_groups=[list(range(NC))],
                    ins=[src.opt()],
                    outs=[dst[:].rearrange("(c r) d -> c r d", c=NC)])

            w1_t = res.tile([128, KT * DW1], f16d)
            w2_t = res.tile([128, KT * DW2], f16d)
            for k in range(KT):
                nc.sync.dma_start(out=w1_t[:, k * DW1:(k + 1) * DW1],
                                  in_=w1[k * 128:(k + 1) * 128, :])
                nc.sync.dma_start(out=w2_t[:, k * DW2:(k + 1) * DW2],
                                  in_=w2[k * 128:(k + 1) * 128, :])
            cilo_t = res.tile([128, NBPC * cpl * 8], i16d)
            cihi_t = res.tile([128, NBPC * cph * 8], i16d)
            nc.sync.dma_start(out=cilo_t[:], in_=cilo[:, :])
            nc.sync.dma_start(out=cihi_t[:], in_=cihi[:, :])
            ident = res.tile([128, 128], f16d)
            make_identity(nc, ident[:])
            gct = res.tile([1, NBPC * (cpl + cph)], mybir.dt.int32)
            nc.sync.dma_start(out=gct[:], in_=gcnt[0:1, :])
            gcnt_t[0] = gct
            fd_sb = res.tile([128, NBPC * 8], f16d)
            fd2_sb = res.tile([128, NBPC], f16d)

            # ---------------- Phase A ----------------
            with nc.named_scope("phaseA"), \
                 tc.tile_pool(name="pa", bufs=5) as pa, \
                 tc.tile_pool(name="ppa", bufs=2, space="PSUM") as ppa:
                def issue_a(nt):
                    xk4 = pa.tile([128, KT * 128], f16d, tag="xk4")
                    nc.sync.dma_start(out=xk4[:], in_=xt[nt, :, :])
                    return xk4
                PFA = 3
                penda = [issue_a(t) for t in range(PFA)]
                for nt in range(NBPC):
                    psA = ppa.tile([128, 512], f32d, tag="psA")
                    psB = ppa.tile([128, 16], f32d, tag="psB")
                    xk4 = penda.pop(0)
                    if nt + PFA < NBPC:
                        penda.append(issue_a(nt + PFA))
                    for k in range(KT):
                        nc.tensor.matmul(out=psA[:],
                                         lhsT=xk4[:, k * 128:(k + 1) * 128],
                                         rhs=w1_t[:, k * DW1:k * DW1 + 512],
                                         start=(k == 0), stop=(k == KT - 1))
                        nc.tensor.matmul(out=psB[:],
                                         lhsT=xk4[:, k * 128:(k + 1) * 128],
                                         rhs=w1_t[:, k * DW1 + 512:(k + 1) * DW1],
                                         start=(k == 0), stop=(k == KT - 1))
                    whf = pa.tile([128, TW1], f8e3, tag="whf")
                    wh16 = whf[:].bitcast(f16d)      # [128, TW1//2] f16 view
                    nc.vector.tensor_copy(out=whf[:, :512], in_=psA[:])
                    nc.scalar.activation(out=wh16[:, 256:264], in_=psB[:, 8:16],
                                         func=AF.Copy)
                    nc.vector.tensor_tensor(out=wh16[:, 264:272],
                                            in0=psB[:, 8:16],
                                            in1=wh16[:, 256:264],
                                            op=ALU.subtract)
                    nc.vector.tensor_copy(out=fd_sb[:, nt * 8:(nt + 1) * 8],
                                          in_=psB[:, 0:8])
                    if nt < TSPL:
                        r0 = nt * 128
                        nc.scalar.dma_start(out=tab1sA[r0:r0 + 128, 0:544],
                                            in_=whf[:, 0:544])
                    else:
                        r0 = (nt - TSPL) * 128
                        nc.scalar.dma_start(out=tab1sB[r0:r0 + 128, 0:544],
                                            in_=whf[:, 0:544])
                    if nt == TSPL - 1:
                        with nc.named_scope("ag1"):
                            ag_sect(tab1sA, tab1a)
                    if nt == NBPC - 1:
                        with nc.named_scope("ag1"):
                            ag_sect(tab1sB, tab1b)

            # ---------------- Phase B ----------------
            # 2-stage software pipeline: stage1(bl) = logits+R (DVE/ACT +
            # small PE), stage2(bl) = scatter/elu/layer-2 (big PE + DVE/ACT).
            # Interleaving stage1(i) with stage2(i-1) keeps every in-order
            # engine queue supplied with ready work (no head-of-line stalls).
            with nc.named_scope("phaseB"), \
                 tc.tile_pool(name="pb", bufs=2) as pb, \
                 tc.tile_pool(name="pr", bufs=3) as pr, \
                 tc.tile_pool(name="pgg", bufs=6) as pgg, \
                 tc.tile_pool(name="pgo", bufs=6) as pgo, \
                 tc.tile_pool(name="ppb", bufs=2, space="PSUM") as ppb, \
                 tc.tile_pool(name="ppf", bufs=3, space="PSUM") as ppf, \
                 tc.tile_pool(name="ppt", bufs=2, space="PSUM") as ppt:
                def issue_b(bl):
                    G = pgg.tile([128, cpe * TW1], f8e3, tag="G")
                    OH = pgo.tile([128, cpe * 128], ohd, tag="OH")
                    OHT = pgo.tile([128, cpe * 128], ohd, tag="OHT")
                    nc.sync.dma_start(out=OHT[:], in_=ohtT[bl, :, :])
                    nc.sync.dma_start(out=OH[:], in_=ohhT[bl, :, :])
                    load_counts(bl)
                    gather_sect(G[:, :cpl * TW1], tab1a[:, :],
                                cilo_t, bl * cpl * 8, cpl, TW1, 0)
                    gather_sect(G[:, cpl * TW1:], tab1b[:, :],
                                cihi_t, bl * cph * 8, cph, TW1, NPL)
                    return G, OH, OHT

                PF = 3
                pend = [issue_b(b) for b in range(PF)]

                def stage1(bl):
                    G, OH, OHT = pend.pop(0)
                    if bl + PF < NBPC:
                        pend.append(issue_b(bl + PF))
                    aux = ppf.tile([128, 8 + cpe * 8 + DW2], f32d, tag="aux")
                    pfd = aux[:, 8:8 + cpe * 8]
                    for c in range(cpe):
                        nc.tensor.matmul(out=pfd[:, c * 8:(c + 1) * 8],
                                         lhsT=OHT[:, c * 128:(c + 1) * 128],
                                         rhs=fd_sb[:, bl * 8:(bl + 1) * 8],
                                         start=True, stop=True)
                    e1 = pb.tile([128, cpe * 8], f32d, tag="e1")
                    p16 = pr.tile([128, cpe * 8], f16d, tag="p16")
                    Gr = G[:].rearrange("p (c d) -> p c d", d=TW1)
                    Gf = G[:].bitcast(f16d).rearrange("p (c d) -> p c d",
                                                      d=TW1 // 2)
                    nc.vector.tensor_tensor(
                        out=e1[:].rearrange("p (c f) -> p c f", c=cpe),
                        in0=Gf[:, :, 256:264],
                        in1=Gf[:, :, 264:272], op=ALU.add)
                    nc.vector.tensor_tensor(out=e1[:], in0=e1[:], in1=pfd,
                                            op=ALU.add)
                    if use_prelu:
                        nc.scalar.activation(out=e1[:], in_=e1[:],
                                             func=AF.Prelu, alpha=ALPHA)
                    else:
                        lr = pb.tile([128, cpe * 8], f32d, tag="lr")
                        nc.vector.tensor_scalar_mul(lr[:], e1[:], ALPHA)
                        nc.vector.tensor_tensor(out=e1[:], in0=e1[:],
                                                in1=lr[:], op=ALU.max)
                    nc.scalar.activation(out=p16[:], in_=e1[:], func=AF.Exp)
                    # R[p, c, f, h] = Wh[p, c, f, h] * p16[p, c, h].
                    # Wh is fp8 (1-byte) which forbids DVE 2x mode, so split:
                    # chunks [0, CSPL) are upcast to f16 on the Scalar engine
                    # then multiplied at 2x on DVE; chunks [CSPL, cpe) are
                    # multiplied directly from fp8 at 1x. CSPL balances the
                    # two engines.
                    R = pr.tile([128, cpe * 512], f16d, tag="R")
                    if CSPL > 0:
                        nc.scalar.activation(
                            out=R[:, :CSPL * 512], in_=Gr[:, 0:CSPL, 0:512],
                            func=AF.Copy)
                        nc.vector.tensor_tensor(
                            out=R[:, :CSPL * 512]
                                .rearrange("p (c f h) -> p c f h", f=64, h=8),
                            in0=R[:, :CSPL * 512]
                                .rearrange("p (c f h) -> p c f h", f=64, h=8),
                            in1=p16[:, :CSPL * 8]
                                .rearrange("p (c o h) -> p c o h", o=1, h=8)
                                .to_broadcast([128, CSPL, 64, 8]),
                            op=ALU.mult)
                    if CSPL < cpe:
                        nc.vector.tensor_tensor(
                            out=R[:, CSPL * 512:]
                                .rearrange("p (c f h) -> p c f h", f=64, h=8),
                            in0=Gr[:, CSPL:, 0:512]
                                .rearrange("p c (f h) -> p c f h", f=64),
                            in1=p16[:, CSPL * 8:]
                                .rearrange("p (c o h) -> p c o h", o=1, h=8)
                                .to_broadcast([128, cpe - CSPL, 64, 8]),
                            op=ALU.mult)
                    return (bl, OH, aux, R, p16)

                def stage2(st):
                    bl, OH, aux, R, p16 = st
                    pden = aux[:, 0:8]
                    ps2 = aux[:, 8 + cpe * 8:8 + cpe * 8 + DW2]
                    pnum = ppb.tile([128, 512], f32d, tag="pnum")
                    for i in range(cpe):
                        nc.tensor.matmul(out=pnum[:],
                                         lhsT=OH[:, i * 128:(i + 1) * 128],
                                         rhs=R[:, i * 512:(i + 1) * 512],
                                         start=(i == 0), stop=(i == cpe - 1))
                        nc.tensor.matmul(out=pden,
                                         lhsT=OH[:, i * 128:(i + 1) * 128],
                                         rhs=p16[:, i * 8:(i + 1) * 8],
                                         start=(i == 0), stop=(i == cpe - 1))
                    dcl = pb.tile([128, 8], f32d, tag="dcl")
                    nc.vector.tensor_scalar_max(dcl[:], pden, 1e-30)
                    nc.vector.reciprocal(out=dcl[:], in_=dcl[:])
                    ha = pb.tile([128, 512], f16d, tag="ha")
                    nc.vector.tensor_tensor(
                        out=ha[:].rearrange("p (f h) -> p f h", f=64),
                        in0=pnum[:].rearrange("p (f h) -> p f h", f=64),
                        in1=dcl[:].rearrange("p (o h) -> p o h", o=1)
                            .to_broadcast([128, 64, 8]),
                        op=ALU.mult)
                    # elu: h16 = max(ha, min(e^ha, 1) - 1)  (exp monotone)
                    hm = pb.tile([128, 512], f16d, tag="hm")
                    nc.scalar.activation(out=hm[:], in_=ha[:], func=AF.Exp)
                    nc.vector.tensor_scalar(out=hm[:], in0=hm[:],
                                            scalar1=1.0, scalar2=-1.0,
                                            op0=ALU.min, op1=ALU.add)
                    h16 = pb.tile([128, 512], f16d, tag="h16")
                    nc.vector.tensor_tensor(out=h16[:], in0=hm[:], in1=ha[:],
                                            op=ALU.max)
                    for k in range(KT):
                        pt = ppt.tile([128, 128], f16d, tag="pt")
                        nc.tensor.transpose(out=pt[:],
                                            in_=h16[:, k * 128:(k + 1) * 128],
                                            identity=ident[:])
                        ht = pb.tile([128, 128], f16d, tag="ht")
                        if k % 2 == 0:
                            nc.vector.tensor_copy(out=ht[:], in_=pt[:])
                        else:
                            nc.scalar.activation(out=ht[:], in_=pt[:],
                                                 func=AF.Copy)
                        nc.tensor.matmul(out=ps2, lhsT=ht[:],
                                         rhs=w2_t[:, k * DW2:(k + 1) * DW2],
                                         start=(k == 0), stop=(k == KT - 1))
                    t2 = pb.tile([128, DW2], f16d, tag="t2")
                    nc.scalar.activation(out=t2[:, 0:64], in_=ps2[:, 0:64],
                                         func=AF.Copy)
                    nc.scalar.activation(out=t2[:, 64:65], in_=ps2[:, 65:66],
                                         func=AF.Copy)
                    nc.vector.tensor_tensor(out=t2[:, 65:66], in0=ps2[:, 65:66],
                                            in1=t2[:, 64:65], op=ALU.subtract)
                    nc.scalar.activation(out=fd2_sb[:, bl:bl + 1],
                                         in_=ps2[:, 64:65], func=AF.Copy)
                    if bl < TSPL:
                        r0 = bl * 128
                        nc.scalar.dma_start(out=tab2sA[r0:r0 + 128, 0:DW2],
                                            in_=t2[:])
                    else:
                        r0 = (bl - TSPL) * 128
                        nc.scalar.dma_start(out=tab2sB[r0:r0 + 128, 0:DW2],
                                            in_=t2[:])
                    if bl == TSPL - 1:
                        with nc.named_scope("ag2"):
                            ag_sect(tab2sA, tab2a)
                    if bl == NBPC - 1:
                        with nc.named_scope("ag2"):
                            ag_sect(tab2sB, tab2b)

                live = []
                for bl in range(NBPC):
                    live.append(stage1(bl))
                    if len(live) > 1:
                        stage2(live.pop(0))
                stage2(live.pop(0))

            # ---------------- Phase C ----------------
            with nc.named_scope("phaseC"), \
                 tc.tile_pool(name="pc", bufs=2) as pc, \
                 tc.tile_pool(name="pr2", bufs=3) as pr2, \
                 tc.tile_pool(name="pg2", bufs=6) as pg2, \
                 tc.tile_pool(name="pgo2", bufs=6) as pgo2, \
                 tc.tile_pool(name="ppc", bufs=3, space="PSUM") as ppc:
                def issue_c(bl):
                    G2 = pg2.tile([128, cpe * TW2], f16d, tag="G2")
                    OH2 = pgo2.tile([128, cpe * 128], ohd, tag="OH2")
                    OH2T = pgo2.tile([128, cpe * 128], ohd, tag="OH2T")
                    nc.sync.dma_start(out=OH2[:], in_=ohhT[bl, :, :])
                    nc.sync.dma_start(out=OH2T[:], in_=ohtT[bl, :, :])
                    load_counts(bl)
                    gather_sect2(G2[:, :cpl * TW2], tab2a[:, :],
                                 cilo_t, bl * cpl * 8, cpl, TW2, 0)
                    gather_sect2(G2[:, cpl * TW2:], tab2b[:, :],
                                 cihi_t, bl * cph * 8, cph, TW2, NPL)
                    return G2, OH2, OH2T

                PF = 3
                pend2 = [issue_c(b) for b in range(PF)]

                def stage1c(bl):
                    G2, OH2, OH2T = pend2.pop(0)
                    if bl + PF < NBPC:
                        pend2.append(issue_c(bl + PF))
                    pfd2 = ppc.tile([128, cpe], f32d, tag="pfd2")
                    for c in range(cpe):
                        nc.tensor.matmul(out=pfd2[:, c:c + 1],
                                         lhsT=OH2T[:, c * 128:(c + 1) * 128],
                                         rhs=fd2_sb[:, bl:bl + 1],
                                         start=True, stop=True)
                    e2 = pc.tile([128, cpe], f32d, tag="e2")
                    p2 = pr2.tile([128, cpe], f16d, tag="p2")
                    G2r = G2[:].rearrange("p (c d) -> p c d", d=TW2)
                    nc.vector.tensor_tensor(
                        out=e2[:].rearrange("p (c o) -> p c o", o=1),
                        in0=G2r[:, :, 64:65],
                        in1=G2r[:, :, 65:66], op=ALU.add)
                    nc.vector.tensor_tensor(
                        out=e2[:], in0=e2[:], in1=pfd2[:], op=ALU.add)
                    if use_prelu:
                        nc.scalar.activation(out=e2[:], in_=e2[:],
                                             func=AF.Prelu, alpha=ALPHA)
                    else:
                        lr2 = pc.tile([128, cpe], f32d, tag="lr2")
                        nc.vector.tensor_scalar_mul(lr2[:], e2[:], ALPHA)
                        nc.vector.tensor_tensor(out=e2[:], in0=e2[:],
                                                in1=lr2[:], op=ALU.max)
                    nc.scalar.activation(out=p2[:], in_=e2[:], func=AF.Exp)
                    R2 = pr2.tile([128, cpe * 65], f16d, tag="R2")
                    R2r = R2[:].rearrange("p (c d) -> p c d", d=65)
                    nc.vector.tensor_tensor(
                        out=R2r[:, :, 0:64],
                        in0=G2r[:, :, 0:64],
                        in1=p2[:].to_broadcast([128, cpe, 64]),
                        op=ALU.mult)
                    nc.vector.tensor_copy(
                        out=R2r[:, :, 64:65],
                        in_=p2[:].rearrange("p (c o) -> p c o", o=1))
                    return (bl, OH2, R2)

                def stage2c(st):
                    bl, OH2, R2 = st
                    rows = slice(bl * 128, (bl + 1) * 128)
                    ps3 = ppc.tile([128, 65], f32d, tag="ps3")
                    for i in range(cpe):
                        nc.tensor.matmul(out=ps3[:],
                                         lhsT=OH2[:, i * 128:(i + 1) * 128],
                                         rhs=R2[:, i * 65:(i + 1) * 65],
                                         start=(i == 0), stop=(i == cpe - 1))
                    d2c = pc.tile([128, 1], f32d, tag="d2c")
                    nc.vector.tensor_scalar_max(d2c[:], ps3[:, 64:65], 1e-30)
                    nc.vector.reciprocal(out=d2c[:], in_=d2c[:])
                    o = pc.tile([128, 64], f32d, tag="o")
                    nc.vector.tensor_tensor(
                        out=o[:].rearrange("p (c f) -> p c f", c=1),
                        in0=ps3[:, 0:64].rearrange("p (c f) -> p c f", c=1),
                        in1=d2c[:].to_broadcast([128, 1, 64]),
                        op=ALU.mult)
                    nc.scalar.dma_start(out=out[rows, :], in_=o[:])

                live2 = []
                for bl in range(NBPC):
                    live2.append(stage1c(bl))
                    if len(live2) > 1:
                        stage2c(live2.pop(0))
                stage2c(live2.pop(0))

    nc.compile()
    _BUILD_CACHE[key] = nc
    return nc


def kernel(**inputs):
    global LAST_EXEC_NS, LAST_RESULTS
    x = inputs["x"].astype(np.float32)
    row = inputs["row"].astype(np.int64)
    col = inputs["col"].astype(np.int64)
    W, a = inputs["W"].astype(np.float32), inputs["a"].astype(np.float32)
    W_out = inputs["W_out"].astype(np.float32)
    a_out = inputs["a_out"].astype(np.float32)

    cilo, cihi, oht, ohh, gcnt, cpl, cph = _preprocess(row, col)

    # head-MINOR (f,h) feature order for layer-1 Wh and layer-2 rows
    W_cat = np.stack([W[h] for h in range(NHEADS)], axis=-1)  # [in, f, h]
    W_cat = W_cat.reshape(NFEAT, NHID * NHEADS)
    WA_dst = np.stack([W[h] @ a[h, :NHID] for h in range(NHEADS)], 1)
    WA_src = np.stack([W[h] @ a[h, NHID:] for h in range(NHEADS)], 1)
    w1_np = np.concatenate([W_cat, WA_dst, WA_src], 1).astype(np.float16)
    w2full = np.concatenate([W_out, (W_out @ a_out[:NCLASS])[:, None],
                             (W_out @ a_out[NCLASS:])[:, None]], 1)
    idx = np.arange(NHID * NHEADS)
    perm = (idx % NHEADS) * NHID + idx // NHEADS   # (f,h) -> h*64+f
    w2_np = w2full[perm, :].astype(np.float16)

    x_pad = np.zeros((NPAD, NFEAT), np.float16)
    x_pad[:N] = x

    nc = _build(cpl, cph)

    fp8 = bool(int(os.environ.get("GAT_FP8_OH", "1")))
    ohdt = mybir.dt.np(f8d) if fp8 else np.float16
    in_maps = []
    for c in range(NC):
        xs = x_pad[c * SHARD:(c + 1) * SHARD]            # [6272, 512]
        xt = (xs.reshape(NBPC, 128, KT, 128)             # [nt, n, k, f]
                .transpose(0, 3, 2, 1)                   # [nt, f, k, n]
                .reshape(NBPC, 128, KT * 128)).copy()
        in_maps.append({"xt": xt, "w1": w1_np, "w2": w2_np,
                        "cilo": cilo[c], "cihi": cihi[c],
                        "gcnt": gcnt[c],
                        "ohtT": oht[c].astype(ohdt),
                        "ohhT": ohh[c].astype(ohdt)})

    trace = bool(int(os.environ.get("GAT_TRACE", "0")))
    res = run_bass_kernel_spmd(nc, in_maps, list(range(NC)), trace=trace,
                               trace_cores=list(range(NC)) if trace else None)
    LAST_EXEC_NS = res.exec_time_ns
    LAST_RESULTS = res
    outs = [res.results[c]["out"] for c in range(NC)]
    return np.concatenate(outs, 0)[:N].astype(np.float32)
